# revision 1
# baseline (speedup 1.0000x reference)
"""Multi-head attention Trainium2 kernel (nn_MultiHeadAttention_86423331930281).

Self-contained: builds a Bass/Tile SPMD kernel, data-parallel over batch
(B=8 -> one batch element per NeuronCore), runs on cores 0-7 via
run_bass_kernel_spmd, returns the full [8, 1024, 1024] output.

Per-core algorithm (S=1024, D=1024, H=16, E=64):
  - transpose q/k/v (PE+identity) -> qT/kT/vT [d, s]
  - V-proj:  V[t, he] = vT.T @ Wv  (fp32r), stored as V1 [t, h, 65] with a
    trailing ones column per head (gives softmax denominators for free)
  - per head-pair m: Q/K-proj -> QT/KT [he_pair=128, s] (bf16),
    scoresT = KT_h^T-slices @ QT_h  (K=64 contraction, 2 heads row-packed),
    exp on ScalarE out of PSUM (scale=1/32 folded in) -> P [t, s],
    attendedT[e|sum, s] += [V_h|1].T @ P  accumulated over t in PSUM (fp32r)
  - batched reciprocal of all denominators, broadcast via DRAM round trip,
    normalize attT (bf16), FC: out = attT.T @ WoT + bo (Wo transposed on PE)
"""

import numpy as np
from contextlib import ExitStack

import concourse.bass as bass
import concourse.mybir as mybir
import concourse.tile as tile
from concourse.bass_utils import run_bass_kernel_spmd
from concourse.masks import make_identity

P = 128
S = 1024          # sequence length
DK = 1024         # qkv input dim
H = 16            # heads
E = 64            # per-head dim
HE = H * E        # 1024
OUT = 1024        # output dim
NT = S // P       # 8 s/t tiles
NK = DK // P      # 8 contraction tiles
NM = H // 2       # 8 head pairs
F32 = mybir.dt.float32
F32R = mybir.dt.float32r
BF16 = mybir.dt.bfloat16
AF = mybir.ActivationFunctionType
ALU = mybir.AluOpType
SCALE = 1.0 / 32.0  # 1/sqrt(DK)


def _r(x):
    """bitcast fp32 AP to fp32r for full-rate matmul"""
    return x.bitcast(F32R)


def _legalize_matmul_waits(nc):
    """This walrus build allows only ONE sync-wait command per Matmult.
    Move all but the last wait of any multi-wait matmul onto freshly
    inserted PE nops immediately before it — same engine queue, so the
    blocking semantics are identical."""
    SKIP = ("NoOp", "Br", "Halt", "Sem", "Event")
    k = 0
    for f in nc.m.functions:
        for b in f.blocks:
            out = []
            for inst in b.instructions:
                si = getattr(inst, "sync_info", None)
                tname = type(inst).__name__
                if (not any(s in tname for s in SKIP) and si is not None
                        and si.on_wait and len(si.on_wait) > 1):
                    waits = list(si.on_wait)
                    for w in waits[:-1]:
                        nop = mybir.InstNoOp(
                            name=f"legalize-nop-{k}", ins=[], outs=[])
                        k += 1
                        nop.engine = inst.engine
                        nop.sync_info = mybir.SyncInfo(
                            on_wait=[w], on_update=[])
                        out.append(nop)
                    inst.sync_info = mybir.SyncInfo(
                        on_wait=[waits[-1]], on_update=list(si.on_update))
                out.append(inst)
            b.instructions[:] = out
    return k


def build(legalize=True):
    nc = bass.Bass()
    q_d = nc.dram_tensor("q", (S, DK), F32, kind="ExternalInput")
    k_d = nc.dram_tensor("k", (S, DK), F32, kind="ExternalInput")
    v_d = nc.dram_tensor("v", (S, DK), F32, kind="ExternalInput")
    wq_d = nc.dram_tensor("wq", (H, DK, E), F32, kind="ExternalInput")
    wk_d = nc.dram_tensor("wk", (H, DK, E), F32, kind="ExternalInput")
    wv_d = nc.dram_tensor("wv", (H, DK, E), F32, kind="ExternalInput")
    wo_d = nc.dram_tensor("wo", (OUT, HE), F32, kind="ExternalInput")
    bo_d = nc.dram_tensor("bo", (OUT,), F32, kind="ExternalInput")
    out_d = nc.dram_tensor("out", (S, OUT), F32, kind="ExternalOutput")
    recip_d = nc.dram_tensor("recip_scratch", (H, S), BF16, kind="Internal")

    # [h, d, e] viewed as [di, ko, h, e] so partition = inner contraction dim
    wq_v = wq_d.rearrange("h (ko ki) e -> ki ko h e", ki=P)
    wk_v = wk_d.rearrange("h (ko ki) e -> ki ko h e", ki=P)
    wv_v = wv_d.rearrange("h (ko ki) e -> ki ko h e", ki=P)

    with tile.TileContext(nc) as tc, ExitStack() as ctx:
        const = ctx.enter_context(tc.tile_pool(name="const", bufs=1))
        src = ctx.enter_context(tc.tile_pool(name="src", bufs=3))
        xTf = ctx.enter_context(tc.tile_pool(name="xTf", bufs=NK))
        xTb = ctx.enter_context(tc.tile_pool(name="xTb", bufs=2 * NK))
        woTp = ctx.enter_context(tc.tile_pool(name="woTp", bufs=NK))
        v1p = ctx.enter_context(tc.tile_pool(name="v1p", bufs=NT))
        ps = ctx.enter_context(tc.tile_pool(name="ps", bufs=2, space="PSUM"))

        ident = const.tile([P, P], F32, name="ident")
        make_identity(nc, ident)
        ident_bf = const.tile([P, P], BF16, name="ident_bf")
        nc.vector.tensor_copy(ident_bf[:], ident[:])
        bo_bc = const.tile([P, OUT], F32, name="bo_bc")
        nc.sync.dma_start(bo_bc[:], bo_d[None, :].to_broadcast((P, OUT)))
        ones_h = const.tile([P, H], F32, name="ones_h")
        nc.gpsimd.memset(ones_h[:], 1.0)
        sums_all = [const.tile([H // 2, S], F32, name=f"sums_all{i}")
                    for i in range(2)]
        recip_bf = [const.tile([H // 2, S], BF16, name=f"recip_bf{i}")
                    for i in range(2)]

        def transpose_mat(mat_d, name, dt, srcb_scalar=False, tpool=None,
                          pool=None):
            """mat [S, DK] fp32 -> 8 tiles [P, S] of mat.T (tile j = rows j*128..)

            The srcb pass-through both absorbs the multi-queue DMA wait and
            (for bf16) does the downcast; evacuation stays on DVE because the
            BIR verifier only accepts DVE writes as fp32r rounding.
            """
            cast_bf = dt == BF16
            if pool is not None:
                tp, tag = pool, "woT"
            elif cast_bf:
                tp, tag = xTb, "xTb"
            else:
                tp, tag = xTf, "xTf"
            tiles = [tp.tile([P, S], dt, name=f"{name}{j}", tag=tag)
                     for j in range(NK)]
            tdt = BF16 if cast_bf else F32
            idt = ident_bf if cast_bf else ident
            dma_engs = [nc.sync, nc.scalar]
            for r in range(NT):
                if cast_bf:
                    # gpsimd DMAs cast in flight: f32 DRAM -> bf16 SBUF
                    stb = src.tile([P, DK], BF16, tag="srcb",
                                   name=f"{name}_srcb{r}")
                    nc.gpsimd.dma_start(stb[:], mat_d[r * P:(r + 1) * P, :])
                else:
                    st = src.tile([P, DK], F32, tag="src", name=f"{name}_src{r}")
                    dma_engs[r % len(dma_engs)].dma_start(
                        st[:], mat_d[r * P:(r + 1) * P, :])
                    stb = st
                for j in range(NK):
                    if tpool is not None:
                        pt_ = tpool.tile([P, P], tdt, tag="tps",
                                         name=f"{name}_ps{r}_{j}")
                    else:
                        pt_ = ps.tile([P, S], tdt, tag="ps",
                                      name=f"{name}_ps{r}_{j}")
                    nc.tensor.transpose(pt_[:, :P], stb[:, j * P:(j + 1) * P], idt[:])
                    dst = tiles[j][:, r * P:(r + 1) * P]
                    if cast_bf and (r + j) % 2 == 1:
                        # bf16 isn't fp32r-rounding-constrained: ACT may evac
                        nc.scalar.copy(dst, pt_[:, :P])
                    else:
                        nc.vector.tensor_copy(dst, pt_[:, :P])
            return tiles

        # first PE instruction: absorb the make_identity (gpsimd) wait into
        # a fresh psum slot (no WAR -> single wait)
        dmy0 = ps.tile([2, P], F32, tag="ps", name="ident_dmy")
        nc.tensor.transpose(dmy0[:2, :P], ident[:, 0:2], ident[:])

        ph1 = ExitStack()
        tps = ph1.enter_context(tc.tile_pool(name="tps", bufs=4, space="PSUM"))
        vT = transpose_mat(v_d, "vT", F32R, srcb_scalar=True, tpool=tps)
        qT = transpose_mat(q_d, "qT", BF16, tpool=tps)
        kT = transpose_mat(k_d, "kT", BF16, tpool=tps)
        v1_tiles = []
        with tc.tile_pool(name="wv", bufs=NK) as wvp:
            wv_tiles = []
            for j in range(NK):
                raw = src.tile([P, H, E], F32, tag="src", name=f"wvr{j}")
                (nc.sync if j % 2 == 0 else nc.scalar).dma_start(raw[:], wv_v[:, j])
                wt = wvp.tile([P, H, E], F32R, tag="wv", name=f"wv{j}")
                nc.vector.tensor_copy(wt[:], raw[:])
                wv_tiles.append(wt)
            for i in range(NT):
                pst = ps.tile([P, HE], F32, tag="ps", name=f"vproj{i}")
                for nh in range(2):
                    for j in range(NK):
                        wvf = wv_tiles[j][:].rearrange("p h e -> p (h e)")
                        nc.tensor.matmul(
                            pst[:, nh * 512:(nh + 1) * 512],
                            vT[j][:, i * P:(i + 1) * P],
                            wvf[:, nh * 512:(nh + 1) * 512],
                            start=(j == 0), stop=(j == NK - 1))
                v1 = v1p.tile([P, H, E + 1], F32R, tag="v1", name=f"v1_{i}")
                nc.vector.tensor_copy(v1[:, :, E], ones_h[:])
                nc.vector.tensor_copy(
                    v1[:, :, 0:E], pst[:].rearrange("p (h e) -> p h e", e=E))
                v1_tiles.append(v1)

        ph1.close()

        # ---- phase 2: per head-pair projections + attention
        wsl = ctx.enter_context(tc.tile_pool(name="wsl", bufs=4))
        qtp = ctx.enter_context(tc.tile_pool(name="qtp", bufs=4))
        ptp = ctx.enter_context(tc.tile_pool(name="ptp", bufs=3))
        attp = ctx.enter_context(tc.tile_pool(name="attp", bufs=NM))
        smallp = ctx.enter_context(tc.tile_pool(name="smallp", bufs=2))
        att_ps = ctx.enter_context(
            tc.tile_pool(name="att_ps", bufs=4, space="PSUM"))

        rbcp = ctx.enter_context(tc.tile_pool(name="rbcp", bufs=2))

        def normalize_batch(ms):
            """reciprocal of denominators for pairs in ms, broadcast, scale"""
            batch = ms[0] // (NM // 2)
            h0 = 2 * ms[0]
            nc.vector.reciprocal(sums_all[batch][:], sums_all[batch][:])
            nc.vector.tensor_copy(recip_bf[batch][:], sums_all[batch][:])
            nc.sync.dma_start(recip_d[h0:h0 + H // 2, :], recip_bf[batch][:])
            for m in ms:
                rbc = rbcp.tile([P, S], BF16, tag="rbc", name=f"rbc{m}")
                for hh in range(2):
                    nc.sync.dma_start(
                        rbc[hh * E:(hh + 1) * E, :],
                        recip_d[2 * m + hh][None, :].to_broadcast((E, S)))
                nc.vector.tensor_tensor(
                    attT_tiles[m][:], attT_tiles[m][:], rbc[:], ALU.mult)

        woT = [woTp.tile([P, S], BF16, name=f"woT{j}", tag="woT")
               for j in range(NK)]

        def wo_row(r):
            stb = src.tile([P, DK], BF16, tag="srcb", name=f"wo_srcb{r}")
            nc.gpsimd.dma_start(stb[:], wo_d[r * P:(r + 1) * P, :])
            for j in range(NK):
                pt_ = ps.tile([P, S], BF16, tag="ps", name=f"wo_ps{r}_{j}")
                nc.tensor.transpose(pt_[:, :P], stb[:, j * P:(j + 1) * P],
                                    ident_bf[:])
                nc.vector.tensor_copy(woT[j][:, r * P:(r + 1) * P], pt_[:, :P])

        attT_tiles = []
        for m in range(NM):
            wqm = wsl.tile([P, NK, 2, E], BF16, tag="wsl", name=f"wq{m}")
            wkm = wsl.tile([P, NK, 2, E], BF16, tag="wsl", name=f"wk{m}")
            wqr = src.tile([P, NK, 2, E], F32, tag="src", name=f"wqr{m}")
            wkr = src.tile([P, NK, 2, E], F32, tag="src", name=f"wkr{m}")
            for hh in range(2):
                nc.sync.dma_start(wqr[:, :, hh, :], wq_v[:, :, 2 * m + hh, :])
                nc.gpsimd.dma_start(wkr[:, :, hh, :], wk_v[:, :, 2 * m + hh, :])
            nc.vector.tensor_copy(wqm[:], wqr[:])
            nc.vector.tensor_copy(wkm[:], wkr[:])

            # QT_m / KT_m: [he_pair=128, s=1024], evacuated as bf16
            qkm = []
            for wm, xtiles, nm in ((wqm, qT, "qtm"), (wkm, kT, "ktm")):
                pst = ps.tile([P, S], F32, tag="ps", name=f"{nm}ps{m}")
                for sh in range(2):
                    for j in range(NK):
                        nc.tensor.matmul(
                            pst[:, sh * 512:(sh + 1) * 512],
                            wm[:, j],
                            xtiles[j][:, sh * 512:(sh + 1) * 512],
                            start=(j == 0), stop=(j == NK - 1))
                t = qtp.tile([P, S], BF16, tag="qt", name=f"{nm}{m}")
                nc.vector.tensor_copy(t[:], pst[:])
                qkm.append(t)
            qtm, ktm = qkm

            att_t = {}
            for hh in range(2):
                for sh in range(2):
                    att_t[hh, sh] = att_ps.tile(
                        [E + 1, 512], F32, tag="attps", name=f"att{m}_{hh}_{sh}")
            for j in range(NT):
                for hh in range(2):
                    hs = slice(hh * E, (hh + 1) * E)
                    sc = ps.tile([P, S], F32, tag="ps", name=f"sc{m}_{j}_{hh}")
                    for sh in range(2):
                        nc.tensor.matmul(
                            sc[:, sh * 512:(sh + 1) * 512],
                            ktm[hs, j * P:(j + 1) * P],
                            qtm[hs, sh * 512:(sh + 1) * 512],
                            start=True, stop=True)
                    ptile = ptp.tile([P, S], F32R, tag="pt", name=f"p{m}_{j}_{hh}")
                    nc.scalar.activation(ptile[:], sc[:], AF.Exp, scale=SCALE)
                    for sh in range(2):
                        nc.tensor.matmul(
                            att_t[hh, sh][:],
                            v1_tiles[j][:, 2 * m + hh, :],
                            ptile[:, sh * 512:(sh + 1) * 512],
                            start=(j == 0), stop=(j == NT - 1))

            # evacuate attendedT + denominators (unnormalized, bf16)
            attm = attp.tile([P, S], BF16, tag="attT", name=f"attT{m}")
            attT_tiles.append(attm)
            for hh in range(2):
                for sh in range(2):
                    apt = att_t[hh, sh]
                    stg = smallp.tile([E + 1, 512], F32, tag="stage",
                                      name=f"stg{m}_{hh}_{sh}")
                    nc.vector.tensor_copy(stg[E:E + 1, :], apt[E:E + 1, :])
                    row = (2 * m + hh) % (H // 2)
                    nc.sync.dma_start(
                        sums_all[m // (NM // 2)][row:row + 1,
                                                 sh * 512:(sh + 1) * 512],
                        stg[E:E + 1, :])
                    nc.vector.tensor_copy(
                        attm[hh * E:(hh + 1) * E, sh * 512:(sh + 1) * 512],
                        apt[0:E, :])
            if m == NM // 2 - 1:
                normalize_batch(list(range(NM // 2)))

        # ---- phase 3: transpose Wo, normalize second half, FC
        for r in range(NT):
            wo_row(r)
        normalize_batch(list(range(NM // 2, NM)))

        outp = ctx.enter_context(tc.tile_pool(name="outp", bufs=2))
        for st in range(NT):
            for oh in range(2):
                pso = att_ps.tile([P, 512], F32, tag="attps",
                                  name=f"fc{st}_{oh}")
                for m in range(NM):
                    nc.tensor.matmul(
                        pso[:],
                        attT_tiles[m][:, st * P:(st + 1) * P],
                        woT[m][:, oh * 512:(oh + 1) * 512],
                        start=(m == 0), stop=(m == NM - 1))
                ot = outp.tile([P, 512], F32, tag="out", name=f"out{st}_{oh}")
                nc.vector.tensor_tensor(
                    ot[:], pso[:], bo_bc[:, oh * 512:(oh + 1) * 512],
                    ALU.add)
                nc.sync.dma_start(
                    out_d[st * P:(st + 1) * P, oh * 512:(oh + 1) * 512], ot[:])
    if legalize:
        _legalize_matmul_waits(nc)
    return nc


_NC_CACHE = {}


def _get_nc():
    if "nc" not in _NC_CACHE:
        _NC_CACHE["nc"] = build()
    return _NC_CACHE["nc"]


def kernel(query, key, value, Wq, Wk, Wv, Wo, bo, **run_kwargs):
    query = np.asarray(query, dtype=np.float32)
    key = np.asarray(key, dtype=np.float32)
    value = np.asarray(value, dtype=np.float32)
    Wq = np.ascontiguousarray(np.asarray(Wq, dtype=np.float32))
    Wk = np.ascontiguousarray(np.asarray(Wk, dtype=np.float32))
    Wv = np.ascontiguousarray(np.asarray(Wv, dtype=np.float32))
    Wo = np.ascontiguousarray(np.asarray(Wo, dtype=np.float32))
    bo = np.ascontiguousarray(np.asarray(bo, dtype=np.float32))
    B = query.shape[0]
    assert B == 8, f"expected batch 8, got {B}"

    nc = _get_nc()
    in_maps = []
    for b in range(B):
        in_maps.append({
            "q": np.ascontiguousarray(query[b]),
            "k": np.ascontiguousarray(key[b]),
            "v": np.ascontiguousarray(value[b]),
            "wq": Wq, "wk": Wk, "wv": Wv, "wo": Wo, "bo": bo,
        })
    res = run_bass_kernel_spmd(nc, in_maps, core_ids=list(range(B)),
                               **run_kwargs)
    out = np.stack([r["out"] for r in res.results], axis=0)
    if run_kwargs.get("trace"):
        _NC_CACHE["last_result"] = res
    return out



# revision 2
# speedup vs baseline: 1.0165x; 1.0165x over previous
"""Multi-head attention Trainium2 kernel (nn_MultiHeadAttention_86423331930281).

Self-contained: data-parallel over batch (B=8 -> one batch element per
NeuronCore), runs on cores 0-7 via run_bass_kernel_spmd, returns the full
[8, 1024, 1024] output.

Per-core algorithm (S=1024, D=1024, H=16, E=64), all-bf16 matmul operands:
  - v/q/k: gpsimd cast-load fp32->bf16 two row-blocks per DMA, PE-transpose
    (bf16 identity, 1 cycle/row) into single [128, 8, S] tiles; one-bank
    row-block psum tiles give one evac per row (DVE/ACT alternating)
  - wo: gpsimd cast-load, store to DRAM bf16 scratch, xbar DMA-transpose
    back -> woT [he, out] (entirely off the critical path, SP-issued)
  - wv: gpsimd strided cast-load [ki, ko, h, e]; wq/wk: per-head-pair
    just-in-time gpsimd cast-loads, prefetched 3 pairs ahead
  - V1[t, h, e|1] = vT.T @ Wv with a trailing ones column per head
  - per head-pair: QT/KT [128=2*64, s] = Wq_pair-chunks.T @ qT (8-chunk
    accum in a dedicated psum pool, decoupled from the exp drain)
  - per head: scoresT [t, s] = KT_h-slices.T @ QT_h (K=64), exp on ACT
    (scale=1/32 folded) -> P [t, s] bf16
  - attended in [s, e] orientation with a ONE-HEAD SOFTWARE LAG: the
    previous head's chains att[s, 65] += P[t-chunk, s-chunk].T @ V1 are
    interleaved into the current head's score emission so they never wait
    on the serial exp stream; the 65th column accumulates the softmax
    denominator for free
  - normalize with per-partition reciprocal+multiply (denominator is a
    per-partition scalar in this orientation -- no broadcast round-trip),
    PE re-transpose [s,64]->[64,s] into attT [he, s]
  - FC split: FC1 = attT[m<7].T @ WoT + rank-1 ones.T@bo runs during the
    last head's exp drain, partials parked in SBUF as bf16; FC2 re-injects
    them via an identity matmul on PE, evacs alternate DVE/ACT
"""

import numpy as np
from contextlib import ExitStack

import concourse.bass as bass
import concourse.mybir as mybir
import concourse.tile as tile
from concourse.bass_utils import run_bass_kernel_spmd
from concourse.masks import make_identity

P = 128
S = 1024          # sequence length
DK = 1024         # qkv input dim
H = 16            # heads
E = 64            # per-head dim
HE = H * E        # 1024
OUT = 1024        # output dim
NT = S // P       # 8 s/t tiles
NK = DK // P      # 8 contraction tiles
NM = H // 2       # 8 head pairs
F32 = mybir.dt.float32
BF16 = mybir.dt.bfloat16
AF = mybir.ActivationFunctionType
ALU = mybir.AluOpType
SCALE = 1.0 / 32.0  # 1/sqrt(DK)


def _legalize_matmul_waits(nc):
    """This walrus build allows only ONE sync-wait command per Matmult.
    Move all but the last wait of any multi-wait matmul onto freshly
    inserted PE nops immediately before it — same engine queue, so the
    blocking semantics are identical."""
    SKIP = ("NoOp", "Br", "Halt", "Sem", "Event")
    k = 0
    for f in nc.m.functions:
        for b in f.blocks:
            out = []
            for inst in b.instructions:
                si = getattr(inst, "sync_info", None)
                tname = type(inst).__name__
                if (not any(s in tname for s in SKIP) and si is not None
                        and si.on_wait and len(si.on_wait) > 1):
                    waits = list(si.on_wait)
                    for w in waits[:-1]:
                        nop = mybir.InstNoOp(
                            name=f"legalize-nop-{k}", ins=[], outs=[])
                        k += 1
                        nop.engine = inst.engine
                        nop.sync_info = mybir.SyncInfo(
                            on_wait=[w], on_update=[])
                        out.append(nop)
                    inst.sync_info = mybir.SyncInfo(
                        on_wait=[waits[-1]], on_update=list(si.on_update))
                out.append(inst)
            b.instructions[:] = out
    return k


def build(legalize=True):
    nc = bass.Bass()
    q_d = nc.dram_tensor("q", (S, DK), F32, kind="ExternalInput")
    k_d = nc.dram_tensor("k", (S, DK), F32, kind="ExternalInput")
    v_d = nc.dram_tensor("v", (S, DK), F32, kind="ExternalInput")
    wq_d = nc.dram_tensor("wq", (H, DK, E), F32, kind="ExternalInput")
    wk_d = nc.dram_tensor("wk", (H, DK, E), F32, kind="ExternalInput")
    wv_d = nc.dram_tensor("wv", (H, DK, E), F32, kind="ExternalInput")
    wo_d = nc.dram_tensor("wo", (OUT, HE), F32, kind="ExternalInput")
    bo_d = nc.dram_tensor("bo", (OUT,), F32, kind="ExternalInput")
    out_d = nc.dram_tensor("out", (S, OUT), F32, kind="ExternalOutput")
    wob_d = nc.dram_tensor("wob_scratch", (OUT, HE), BF16, kind="Internal")

    # [h, d, e] viewed as [di, ko, h, e] so partition = inner contraction dim
    wq_v = wq_d.rearrange("h (ko ki) e -> ki ko h e", ki=P)
    wk_v = wk_d.rearrange("h (ko ki) e -> ki ko h e", ki=P)
    wv_v = wv_d.rearrange("h (ko ki) e -> ki ko h e", ki=P)

    with tile.TileContext(nc) as tc, ExitStack() as ctx:
        const = ctx.enter_context(tc.tile_pool(name="const", bufs=1))
        src = ctx.enter_context(tc.tile_pool(name="src", bufs=4))
        xTq = ctx.enter_context(tc.tile_pool(name="xTq", bufs=1))
        woTp = ctx.enter_context(tc.tile_pool(name="woTp", bufs=NK))
        wqkp = ctx.enter_context(tc.tile_pool(name="wqkp", bufs=6))
        v1p = ctx.enter_context(tc.tile_pool(name="v1p", bufs=NT))
        # scores psum: 2 x 2 banks; proj/fc psum: 2 x 1 bank (decoupled so
        # projections never wait on the exp drain tail); the re-transpose
        # collect tiles share the proj slots (same tag, same bank size)
        scp = ctx.enter_context(tc.tile_pool(name="scp", bufs=2, space="PSUM"))
        pjp = ctx.enter_context(tc.tile_pool(name="pjp", bufs=2, space="PSUM"))
        ph1 = ExitStack()
        vTp = ph1.enter_context(tc.tile_pool(name="vTp", bufs=1))
        wvp = ph1.enter_context(tc.tile_pool(name="wvp", bufs=2))


        # ---- load + transpose phase --------------------------------------
        # one [128, NK, S] tile per transposed matrix; chunk j = [:, j, :]
        vTq = vTp.tile([P, NK, S], BF16, name="vT", tag="vT")
        qTq = xTq.tile([P, NK, S], BF16, name="qT", tag="qT")
        kTq = xTq.tile([P, NK, S], BF16, name="kT", tag="kT")
        woT = [woTp.tile([P, S], BF16, name=f"woT{j}", tag="woT")
               for j in range(NK)]

        tpp = ph1.enter_context(tc.tile_pool(name="tpp", bufs=2,
                                             space="PSUM"))

        def emit_casts(mat_d, nm, n=None):
            """cast-load two row-blocks per DMA (halves the SWDGE count)"""
            stbs = []
            for rr in range(n if n is not None else NT // 2):
                stb = src.tile([P, 2, DK], BF16, tag="srcb",
                               name=f"{nm}cast{rr}")
                nc.gpsimd.dma_start(
                    stb[:],
                    mat_d[rr * 2 * P:(rr + 1) * 2 * P, :].rearrange(
                        "(c p) d -> p c d", c=2))
                stbs.append(stb)
            return stbs

        def emit_transposes(stbs, tile, nm):
            """PE-transpose a full row-block into one 1-bank psum tile;
            single evac per row-block (DVE/ACT alternating — GPSIMD cannot
            read PSUM)"""
            for r in range(NT):
                stb = stbs[r // 2]
                c = r % 2
                pt_ = tpp.tile([P, NK, P], BF16, tag="tp", name=f"{nm}ps{r}")
                for j in range(NK):
                    nc.tensor.transpose(
                        pt_[:, j, :], stb[:, c, j * P:(j + 1) * P],
                        ident_bf[:])
                if r % 2 == 0:
                    nc.vector.tensor_copy(
                        tile[:, :, r * P:(r + 1) * P], pt_[:])
                else:
                    nc.scalar.copy(
                        tile[:, :, r * P:(r + 1) * P], pt_[:])

        def pe_transpose(mat_d, tile, nm):
            emit_transposes(emit_casts(mat_d, nm), tile, nm)

        def xs(tile, j):
            """[128, S] view of transposed chunk j"""
            return tile[:, j, :]

        # first v cast goes out before the identity init so data and
        # identity land together for the first transpose
        v_stbs = emit_casts(v_d[0:2 * P, :], "v0", n=1)
        ident = const.tile([P, P], F32, name="ident")
        make_identity(nc, ident)
        ident_bf = const.tile([P, P], BF16, name="ident_bf")
        nc.vector.tensor_copy(ident_bf[:], ident[:])
        v_stbs += emit_casts(v_d[2 * P:, :], "v1", n=3)
        emit_transposes(v_stbs, vTq, "v")

        # wv: strided cast-load [ki, ko, h, e] per contraction chunk
        wv_sb = []
        for half in range(2):
            t = wvp.tile([P, NK // 2, H, E], BF16, tag="wwv",
                         name=f"wvsb{half}")
            for jj in range(NK // 2):
                nc.gpsimd.dma_start(
                    t[:, jj], wv_v[:, half * (NK // 2) + jj])
            wv_sb.append(t)

        def prefetch_w(m):
            """per-pair just-in-time Wq/Wk chunk loads [ki, ko, 2, e]"""
            wqm = wqkp.tile([P, NK, 2, E], BF16, tag="wqk", name=f"wqm{m}")
            wkm = wqkp.tile([P, NK, 2, E], BF16, tag="wqk", name=f"wkm{m}")
            for hh in range(2):
                nc.gpsimd.dma_start(wqm[:, :, hh, :], wq_v[:, :, 2 * m + hh, :])
                nc.gpsimd.dma_start(wkm[:, :, hh, :], wk_v[:, :, 2 * m + hh, :])
            return wqm, wkm

        w_pref = {0: prefetch_w(0)}

        # ---- V projection: V1 [t, h, e|ones] ------------------------------
        v1_tiles = []
        for i in range(NT):
            v1 = v1p.tile([P, H, E + 1], BF16, tag="v1", name=f"v1_{i}")
            nc.gpsimd.memset(v1[:, :, E], 1.0)
            for nh in range(2):
                pst = pjp.tile([P, 512], F32, tag="pj", name=f"vproj{i}_{nh}")
                for j in range(NK):
                    wvf = wv_sb[j // (NK // 2)][:, j % (NK // 2)].rearrange(
                        "p h e -> p (h e)")
                    nc.tensor.matmul(
                        pst[:],
                        xs(vTq, j)[:, i * P:(i + 1) * P],
                        wvf[:, nh * 512:(nh + 1) * 512],
                        start=(j == 0), stop=(j == NK - 1))
                nc.vector.tensor_copy(
                    v1[:, nh * (H // 2):(nh + 1) * (H // 2), 0:E],
                    pst[:].rearrange("p (h e) -> p h e", e=E))
            v1_tiles.append(v1)

        w_pref[1] = prefetch_w(1)
        pe_transpose(q_d, qTq, "q")
        w_pref[2] = prefetch_w(2)
        pe_transpose(k_d, kTq, "k")

        # FC-only constants, emitted after the critical-path loads
        ones_row = const.tile([1, P], BF16, name="ones_row")
        nc.gpsimd.memset(ones_row[:], 1.0)
        bo_bf = const.tile([1, OUT], BF16, name="bo_bf")
        nc.gpsimd.dma_start(bo_bf[:], bo_d[None, :])

        ph1.close()

        # ---- attention (one-head software pipeline lag) -------------------
        qtp = ctx.enter_context(tc.tile_pool(name="qtp", bufs=4))
        ptp = ctx.enter_context(tc.tile_pool(name="ptp", bufs=2 * NT))
        normp = ctx.enter_context(tc.tile_pool(name="normp", bufs=16))
        denp = ctx.enter_context(tc.tile_pool(name="denp", bufs=8))
        attp = ctx.enter_context(tc.tile_pool(name="attp", bufs=NM))
        att_ps = ctx.enter_context(
            tc.tile_pool(name="att_ps", bufs=2, space="PSUM"))

        attT_tiles = [attp.tile([P, S], BF16, tag="attT", name=f"attT{m}")
                      for m in range(NM)]

        # wo: cast-load bf16 (gpsimd, queued after the critical-path loads),
        # store to scratch (SP), xbar DMA-transpose back (SP) — SP is
        # otherwise idle until the out writes; needed only by the FC
        for rr in range(NT // 2):
            stb = src.tile([P, 2, DK], BF16, tag="srcb", name=f"wocast{rr}")
            nc.gpsimd.dma_start(
                stb[:],
                wo_d[rr * 2 * P:(rr + 1) * 2 * P, :].rearrange(
                    "(c p) d -> p c d", c=2))
            for c in range(2):
                r = rr * 2 + c
                nc.sync.dma_start(wob_d[r * P:(r + 1) * P, :], stb[:, c, :])
        for j in range(NK):
            nc.sync.dma_start_transpose(
                woT[j][:], wob_d[:, j * P:(j + 1) * P])

        def emit_att(h, ptiles, si):
            """attended [s-chunk si, e|denom] for head h + normalize"""
            if si % 2 == 0:
                _att_slot[0] = att_ps.tile([P, 2, E + 1], F32, tag="attps",
                                           name=f"att{h}_{si}")
            aps = _att_slot[0][:, si % 2, :]
            for j in range(NT):
                nc.tensor.matmul(
                    aps[0:P, 0:E + 1],
                    ptiles[j][:, si * P:(si + 1) * P],
                    v1_tiles[j][:, h, :],
                    start=(j == 0), stop=(j == NT - 1))
            den = denp.tile([P, 1], F32, tag="den", name=f"den{h}_{si}")
            nc.vector.reciprocal(den[:], aps[0:P, E:E + 1])
            nrm = normp.tile([P, E], BF16, tag="nrm", name=f"nrm{h}_{si}")
            nc.vector.tensor_scalar(nrm[:], aps[0:P, 0:E], den[:], None,
                                    ALU.mult)
            return nrm

        _att_slot = [None]
        pend = None  # (m, hs, nrm list) awaiting re-transpose + evac

        def flush_pend():
            nonlocal pend
            if pend is None:
                return
            pm, phs, ph_, nrms = pend
            tph = pjp.tile([E, S], BF16, tag="pj", name=f"tph{ph_}")
            for si in range(NT):
                nc.tensor.transpose(tph[:, si * P:(si + 1) * P], nrms[si][:],
                                    ident_bf[:])
            nc.vector.tensor_copy(attT_tiles[pm][phs, :], tph[:])
            pend = None

        prev_att = None  # (h, ptiles) whose attended chains interleave next

        for m in range(NM):
            if m + 3 < NM:
                w_pref[m + 3] = prefetch_w(m + 3)
            wqm, wkm = w_pref.pop(m)

            # QT_m / KT_m: [he_pair=128, s=1024], evacuated as bf16
            qkm = []
            for wm, xtiles, lbl in ((wqm, qTq, "qtm"), (wkm, kTq, "ktm")):
                t = qtp.tile([P, S], BF16, tag="qt", name=f"{lbl}{m}")
                for sh in range(2):
                    pst = pjp.tile([P, 512], F32, tag="pj",
                                   name=f"{lbl}ps{m}_{sh}")
                    for j in range(NK):
                        nc.tensor.matmul(
                            pst[:],
                            wm[:, j],
                            xs(xtiles, j)[:, sh * 512:(sh + 1) * 512],
                            start=(j == 0), stop=(j == NK - 1))
                    nc.vector.tensor_copy(t[:, sh * 512:(sh + 1) * 512],
                                          pst[:])
                qkm.append(t)
            qtm, ktm = qkm

            for hh in range(2):
                h = 2 * m + hh
                hs = slice(hh * E, (hh + 1) * E)
                # scoresT + exp -> P_j [t, s] bf16, with the previous head's
                # attended chains interleaved (their exps are already done)
                ptiles = []
                for j in range(NT):
                    pt = ptp.tile([P, S], BF16, tag="pt", name=f"p{h}_{j}")
                    sc = scp.tile([P, S], F32, tag="sc", name=f"sc{h}_{j}")
                    for sh in range(2):
                        nc.tensor.matmul(
                            sc[:, sh * 512:(sh + 1) * 512],
                            ktm[hs, j * P:(j + 1) * P],
                            qtm[hs, sh * 512:(sh + 1) * 512],
                            start=True, stop=True)
                    nc.scalar.activation(pt[:], sc[:], AF.Exp, scale=SCALE)
                    ptiles.append(pt)
                    if prev_att is not None:
                        nrm = emit_att(prev_att[0], prev_att[1], j)
                        prev_att[2].append(nrm)
                if prev_att is not None:
                    ph_, ppt, nrms = prev_att
                    flush_pend()
                    pend = (ph_ // 2, slice((ph_ % 2) * E, (ph_ % 2 + 1) * E),
                            ph_, nrms)
                prev_att = (h, ptiles, [])

        # ---- drain + FC, software-pipelined ------------------------------
        # FC1 (heads of pairs 0..6) runs while the last head's exps drain;
        # its partials (+bias) park in SBUF. The tail is then only the last
        # head's attended, its transposes, and a single-matmul FC2 pass.
        outp = ctx.enter_context(tc.tile_pool(name="outp", bufs=8))
        fc1p = ctx.enter_context(tc.tile_pool(name="fc1p", bufs=2 * NT))
        ph_, ppt, nrms = prev_att
        flush_pend()
        chunks = [(st, oh) for st in range(NT) for oh in range(2)]
        fc1_sb = []
        for ci, (st, oh) in enumerate(chunks):
            pso = pjp.tile([P, 512], F32, tag="pj", name=f"fc1_{st}_{oh}")
            for m in range(NM - 1):
                nc.tensor.matmul(
                    pso[:],
                    attT_tiles[m][:, st * P:(st + 1) * P],
                    woT[m][:, oh * 512:(oh + 1) * 512],
                    start=(m == 0), stop=False)
            # rank-1 bias: ones[1,128].T @ bo[1,512] adds bo to every row,
            # and closes the accumulation group
            nc.tensor.matmul(
                pso[:], ones_row[:], bo_bf[:, oh * 512:(oh + 1) * 512],
                start=False, stop=True)
            t = fc1p.tile([P, 512], BF16, tag="fc1", name=f"fc1sb{st}_{oh}")
            # ACT evac: DVE is busy with the last head's normalize chain
            nc.scalar.copy(t[:], pso[:])
            fc1_sb.append(t)
            if ci % 2 == 0 and ci // 2 < NT:
                nrms.append(emit_att(ph_, ppt, ci // 2))
        pend = (ph_ // 2, slice((ph_ % 2) * E, (ph_ % 2 + 1) * E), ph_, nrms)
        flush_pend()
        for ci, ((st, oh), fc1t) in enumerate(zip(chunks, fc1_sb)):
            # alternate psum pools so the evac never gates the next chunk;
            # the FC1 partial is re-injected on PE via an identity matmul
            pool, tag = (pjp, "pj") if ci % 2 == 0 else (scp, "sc")
            pso = pool.tile([P, 512], F32, tag=tag, name=f"fc2_{st}_{oh}")
            nc.tensor.matmul(
                pso[:],
                attT_tiles[NM - 1][:, st * P:(st + 1) * P],
                woT[NM - 1][:, oh * 512:(oh + 1) * 512],
                start=True, stop=False)
            nc.tensor.matmul(
                pso[:], ident_bf[:], fc1t[:], start=False, stop=True)
            ot = outp.tile([P, 512], F32, tag="out", name=f"out{st}_{oh}")
            if ci % 2 == 0:
                nc.vector.tensor_copy(ot[:], pso[:])
            else:
                nc.scalar.copy(ot[:], pso[:])
            nc.sync.dma_start(
                out_d[st * P:(st + 1) * P, oh * 512:(oh + 1) * 512], ot[:])
    if legalize:
        _legalize_matmul_waits(nc)
    return nc


_NC_CACHE = {}


def _get_nc():
    if "nc" not in _NC_CACHE:
        _NC_CACHE["nc"] = build()
    return _NC_CACHE["nc"]


def kernel(query, key, value, Wq, Wk, Wv, Wo, bo, **run_kwargs):
    query = np.asarray(query, dtype=np.float32)
    key = np.asarray(key, dtype=np.float32)
    value = np.asarray(value, dtype=np.float32)
    Wq = np.ascontiguousarray(np.asarray(Wq, dtype=np.float32))
    Wk = np.ascontiguousarray(np.asarray(Wk, dtype=np.float32))
    Wv = np.ascontiguousarray(np.asarray(Wv, dtype=np.float32))
    Wo = np.ascontiguousarray(np.asarray(Wo, dtype=np.float32))
    bo = np.ascontiguousarray(np.asarray(bo, dtype=np.float32))
    B = query.shape[0]
    assert B == 8, f"expected batch 8, got {B}"

    nc = _get_nc()
    in_maps = []
    for b in range(B):
        in_maps.append({
            "q": np.ascontiguousarray(query[b]),
            "k": np.ascontiguousarray(key[b]),
            "v": np.ascontiguousarray(value[b]),
            "wq": Wq, "wk": Wk, "wv": Wv, "wo": Wo, "bo": bo,
        })
    res = run_bass_kernel_spmd(nc, in_maps, core_ids=list(range(B)),
                               **run_kwargs)
    out = np.stack([r["out"] for r in res.results], axis=0)
    if run_kwargs.get("trace"):
        _NC_CACHE["last_result"] = res
    return out


# revision 3
# speedup vs baseline: 1.0415x; 1.0247x over previous
"""Multi-head attention Trainium2 kernel (nn_MultiHeadAttention_86423331930281).

Self-contained: data-parallel over batch (B=8 -> one batch element per
NeuronCore), runs on cores 0-7 via run_bass_kernel_spmd, returns the full
[8, 1024, 1024] output.

Per-core algorithm (S=1024, D=1024, H=16, E=64), all-bf16 matmul operands:
  - v/q/k: gpsimd cast-load fp32->bf16 two row-blocks per DMA, PE-transpose
    (bf16 identity, 1 cycle/row) into single [128, 8, S] tiles; one-bank
    row-block psum tiles give one evac per row (DVE/ACT alternating)
  - wo: gpsimd cast-load, store to DRAM bf16 scratch, xbar DMA-transpose
    back -> woT [he, out] (entirely off the critical path, SP-issued)
  - wv: gpsimd strided cast-load [ki, ko, h, e]; wq/wk: per-head-pair
    just-in-time gpsimd cast-loads, prefetched 3 pairs ahead
  - V1[t, h, e|1] = vT.T @ Wv with a trailing ones column per head
  - per head-pair: QT/KT [128=2*64, s] = Wq_pair-chunks.T @ qT (8-chunk
    accum in a dedicated psum pool, decoupled from the exp drain)
  - per head: scoresT [t, s] = KT_h-slices.T @ QT_h (K=64), exp on ACT
    (scale=1/32 folded) -> P [t, s] bf16
  - attended in [s, e] orientation with a ONE-HEAD SOFTWARE LAG: the
    previous head's chains att[s, 65] += P[t-chunk, s-chunk].T @ V1 are
    interleaved into the current head's score emission so they never wait
    on the serial exp stream; the 65th column accumulates the softmax
    denominator for free
  - normalize with per-partition reciprocal+multiply (denominator is a
    per-partition scalar in this orientation -- no broadcast round-trip),
    PE re-transpose [s,64]->[64,s] into attT [he, s]
  - FC split: FC1 = attT[m<7].T @ WoT + rank-1 ones.T@bo runs during the
    last head's exp drain, partials parked in SBUF as bf16; FC2 re-injects
    them via an identity matmul on PE, evacs alternate DVE/ACT
"""

import numpy as np
from contextlib import ExitStack

import concourse.bass as bass
import concourse.mybir as mybir
import concourse.tile as tile
from concourse.bass_utils import run_bass_kernel_spmd
from concourse.masks import make_identity

P = 128
S = 1024          # sequence length
DK = 1024         # qkv input dim
H = 16            # heads
E = 64            # per-head dim
HE = H * E        # 1024
OUT = 1024        # output dim
NT = S // P       # 8 s/t tiles
NK = DK // P      # 8 contraction tiles
NM = H // 2       # 8 head pairs
F32 = mybir.dt.float32
BF16 = mybir.dt.bfloat16
AF = mybir.ActivationFunctionType
ALU = mybir.AluOpType
SCALE = 1.0 / 32.0  # 1/sqrt(DK)


def _legalize_matmul_waits(nc):
    """This walrus build allows only ONE sync-wait command per Matmult.
    Move all but the last wait of any multi-wait matmul onto freshly
    inserted PE nops immediately before it — same engine queue, so the
    blocking semantics are identical."""
    SKIP = ("NoOp", "Br", "Halt", "Sem", "Event")
    k = 0
    for f in nc.m.functions:
        for b in f.blocks:
            out = []
            for inst in b.instructions:
                si = getattr(inst, "sync_info", None)
                tname = type(inst).__name__
                if (not any(s in tname for s in SKIP) and si is not None
                        and si.on_wait and len(si.on_wait) > 1):
                    waits = list(si.on_wait)
                    for w in waits[:-1]:
                        nop = mybir.InstNoOp(
                            name=f"legalize-nop-{k}", ins=[], outs=[])
                        k += 1
                        nop.engine = inst.engine
                        nop.sync_info = mybir.SyncInfo(
                            on_wait=[w], on_update=[])
                        out.append(nop)
                    inst.sync_info = mybir.SyncInfo(
                        on_wait=[waits[-1]], on_update=list(si.on_update))
                out.append(inst)
            b.instructions[:] = out
    return k


def build(legalize=True):
    nc = bass.Bass()
    q_d = nc.dram_tensor("q", (S, DK), F32, kind="ExternalInput")
    k_d = nc.dram_tensor("k", (S, DK), F32, kind="ExternalInput")
    v_d = nc.dram_tensor("v", (S, DK), F32, kind="ExternalInput")
    wq_d = nc.dram_tensor("wq", (H, DK, E), F32, kind="ExternalInput")
    wk_d = nc.dram_tensor("wk", (H, DK, E), F32, kind="ExternalInput")
    wv_d = nc.dram_tensor("wv", (H, DK, E), F32, kind="ExternalInput")
    wo_d = nc.dram_tensor("wo", (OUT, HE), F32, kind="ExternalInput")
    bo_d = nc.dram_tensor("bo", (OUT,), F32, kind="ExternalInput")
    out_d = nc.dram_tensor("out", (S, OUT), F32, kind="ExternalOutput")
    wob_d = nc.dram_tensor("wob_scratch", (OUT, HE), BF16, kind="Internal")

    # [h, d, e] viewed as [di, ko, h, e] so partition = inner contraction dim
    wq_v = wq_d.rearrange("h (ko ki) e -> ki ko h e", ki=P)
    wk_v = wk_d.rearrange("h (ko ki) e -> ki ko h e", ki=P)
    wv_v = wv_d.rearrange("h (ko ki) e -> ki ko h e", ki=P)

    with tile.TileContext(nc) as tc, ExitStack() as ctx:
        const = ctx.enter_context(tc.tile_pool(name="const", bufs=1))
        src = ctx.enter_context(tc.tile_pool(name="src", bufs=4))
        xTq = ctx.enter_context(tc.tile_pool(name="xTq", bufs=1))
        woTp = ctx.enter_context(tc.tile_pool(name="woTp", bufs=NK))
        wqkp = ctx.enter_context(tc.tile_pool(name="wqkp", bufs=6))
        v1p = ctx.enter_context(tc.tile_pool(name="v1p", bufs=NT))
        # scores psum: 2 x 2 banks; proj/fc psum: 2 x 1 bank (decoupled so
        # projections never wait on the exp drain tail); the re-transpose
        # collect tiles share the proj slots (same tag, same bank size)
        scp = ctx.enter_context(tc.tile_pool(name="scp", bufs=2, space="PSUM"))
        pjp = ctx.enter_context(tc.tile_pool(name="pjp", bufs=2, space="PSUM"))
        ph1 = ExitStack()
        vTp = ph1.enter_context(tc.tile_pool(name="vTp", bufs=1))
        wvp = ph1.enter_context(tc.tile_pool(name="wvp", bufs=2))


        # ---- load + transpose phase --------------------------------------
        # one [128, NK, S] tile per transposed matrix; chunk j = [:, j, :]
        vTq = vTp.tile([P, NK, S], BF16, name="vT", tag="vT")
        qTq = xTq.tile([P, NK, S], BF16, name="qT", tag="qT")
        kTq = xTq.tile([P, NK, S], BF16, name="kT", tag="kT")
        woT = [woTp.tile([P, S], BF16, name=f"woT{j}", tag="woT")
               for j in range(NK)]

        tpp = ph1.enter_context(tc.tile_pool(name="tpp", bufs=2,
                                             space="PSUM"))

        def emit_casts(mat_d, nm, n=None):
            """cast-load two row-blocks per DMA (halves the SWDGE count)"""
            stbs = []
            for rr in range(n if n is not None else NT // 2):
                stb = src.tile([P, 2, DK], BF16, tag="srcb",
                               name=f"{nm}cast{rr}")
                nc.gpsimd.dma_start(
                    stb[:],
                    mat_d[rr * 2 * P:(rr + 1) * 2 * P, :].rearrange(
                        "(c p) d -> p c d", c=2))
                stbs.append(stb)
            return stbs

        def emit_transposes(stbs, tile, nm):
            """PE-transpose a full row-block into one 1-bank psum tile;
            single evac per row-block (DVE/ACT alternating — GPSIMD cannot
            read PSUM)"""
            for r in range(NT):
                stb = stbs[r // 2]
                c = r % 2
                # alternate with the (still idle) scores pool slots so the
                # evac WAR never paces the transposes
                pool, tg = (tpp, "tp") if r % 2 == 0 else (scp, "sc")
                pt_ = pool.tile([P, NK, P], BF16, tag=tg, name=f"{nm}ps{r}")
                for j in range(NK):
                    nc.tensor.transpose(
                        pt_[:, j, :], stb[:, c, j * P:(j + 1) * P],
                        ident_bf[:])
                if r % 2 == 0:
                    nc.vector.tensor_copy(
                        tile[:, :, r * P:(r + 1) * P], pt_[:])
                else:
                    nc.scalar.copy(
                        tile[:, :, r * P:(r + 1) * P], pt_[:])

        def pe_transpose(mat_d, tile, nm):
            emit_transposes(emit_casts(mat_d, nm), tile, nm)

        def xs(tile, j):
            """[128, S] view of transposed chunk j"""
            return tile[:, j, :]

        # first v cast goes out before the identity init so data and
        # identity land together for the first transpose
        v_stbs = emit_casts(v_d[0:2 * P, :], "v0", n=1)
        ident = const.tile([P, P], F32, name="ident")
        make_identity(nc, ident)
        ident_bf = const.tile([P, P], BF16, name="ident_bf")
        nc.vector.tensor_copy(ident_bf[:], ident[:])
        v_stbs += emit_casts(v_d[2 * P:, :], "v1", n=3)
        emit_transposes(v_stbs, vTq, "v")

        # wv: strided cast-load [ki, ko, h, e] per contraction chunk
        wv_sb = []
        for half in range(2):
            t = wvp.tile([P, NK // 2, H, E], BF16, tag="wwv",
                         name=f"wvsb{half}")
            for jj in range(NK // 2):
                nc.gpsimd.dma_start(
                    t[:, jj], wv_v[:, half * (NK // 2) + jj])
            wv_sb.append(t)

        def prefetch_w(m):
            """per-pair just-in-time Wq/Wk chunk loads [ki, ko, 2, e]"""
            wqm = wqkp.tile([P, NK, 2, E], BF16, tag="wqk", name=f"wqm{m}")
            wkm = wqkp.tile([P, NK, 2, E], BF16, tag="wqk", name=f"wkm{m}")
            for hh in range(2):
                nc.gpsimd.dma_start(wqm[:, :, hh, :], wq_v[:, :, 2 * m + hh, :])
                nc.gpsimd.dma_start(wkm[:, :, hh, :], wk_v[:, :, 2 * m + hh, :])
            return wqm, wkm

        w_pref = {0: prefetch_w(0)}

        # ---- V projection: V1 [t, h, e|ones] ------------------------------
        v1_tiles = []
        for i in range(NT):
            v1 = v1p.tile([P, H, E + 1], BF16, tag="v1", name=f"v1_{i}")
            nc.gpsimd.memset(v1[:, :, E], 1.0)
            for nh in range(2):
                pst = pjp.tile([P, 512], F32, tag="pj", name=f"vproj{i}_{nh}")
                for j in range(NK):
                    wvf = wv_sb[j // (NK // 2)][:, j % (NK // 2)].rearrange(
                        "p h e -> p (h e)")
                    nc.tensor.matmul(
                        pst[:],
                        xs(vTq, j)[:, i * P:(i + 1) * P],
                        wvf[:, nh * 512:(nh + 1) * 512],
                        start=(j == 0), stop=(j == NK - 1))
                nc.vector.tensor_copy(
                    v1[:, nh * (H // 2):(nh + 1) * (H // 2), 0:E],
                    pst[:].rearrange("p (h e) -> p h e", e=E))
            v1_tiles.append(v1)

        w_pref[1] = prefetch_w(1)
        pe_transpose(q_d, qTq, "q")
        w_pref[2] = prefetch_w(2)
        pe_transpose(k_d, kTq, "k")

        # FC-only constants, emitted after the critical-path loads
        ones_row = const.tile([1, P], BF16, name="ones_row")
        nc.gpsimd.memset(ones_row[:], 1.0)
        bo_bf = const.tile([1, OUT], BF16, name="bo_bf")
        nc.gpsimd.dma_start(bo_bf[:], bo_d[None, :])

        ph1.close()

        # ---- attention (one-head software pipeline lag) -------------------
        qtp = ctx.enter_context(tc.tile_pool(name="qtp", bufs=4))
        ptp = ctx.enter_context(tc.tile_pool(name="ptp", bufs=2 * NT))
        normp = ctx.enter_context(tc.tile_pool(name="normp", bufs=16))
        denp = ctx.enter_context(tc.tile_pool(name="denp", bufs=8))
        attp = ctx.enter_context(tc.tile_pool(name="attp", bufs=NM))
        att_ps = ctx.enter_context(
            tc.tile_pool(name="att_ps", bufs=2, space="PSUM"))

        attT_tiles = [attp.tile([P, S], BF16, tag="attT", name=f"attT{m}")
                      for m in range(NM)]

        # wo: cast-load bf16 (gpsimd, queued after the critical-path loads),
        # store to scratch (SP), xbar DMA-transpose back (SP) — SP is
        # otherwise idle until the out writes; needed only by the FC
        for rr in range(NT // 2):
            stb = src.tile([P, 2, DK], BF16, tag="srcb", name=f"wocast{rr}")
            nc.gpsimd.dma_start(
                stb[:],
                wo_d[rr * 2 * P:(rr + 1) * 2 * P, :].rearrange(
                    "(c p) d -> p c d", c=2))
            for c in range(2):
                r = rr * 2 + c
                nc.sync.dma_start(wob_d[r * P:(r + 1) * P, :], stb[:, c, :])
        for j in range(NK):
            nc.sync.dma_start_transpose(
                woT[j][:], wob_d[:, j * P:(j + 1) * P])

        def emit_att(h, ptiles, si):
            """attended [s-chunk si, e|denom] for head h + normalize"""
            if si % 2 == 0:
                _att_slot[0] = att_ps.tile([P, 2, E + 1], F32, tag="attps",
                                           name=f"att{h}_{si}")
            aps = _att_slot[0][:, si % 2, :]
            for j in range(NT):
                nc.tensor.matmul(
                    aps[0:P, 0:E + 1],
                    ptiles[j][:, si * P:(si + 1) * P],
                    v1_tiles[j][:, h, :],
                    start=(j == 0), stop=(j == NT - 1))
            den = denp.tile([P, 1], F32, tag="den", name=f"den{h}_{si}")
            nc.vector.reciprocal(den[:], aps[0:P, E:E + 1])
            nrm = normp.tile([P, E], BF16, tag="nrm", name=f"nrm{h}_{si}")
            nc.vector.tensor_scalar(nrm[:], aps[0:P, 0:E], den[:], None,
                                    ALU.mult)
            return nrm

        _att_slot = [None]
        pend = None  # (m, hs, nrm list) awaiting re-transpose + evac

        def flush_pend():
            nonlocal pend
            if pend is None:
                return
            pm, phs, ph_, nrms = pend
            tph = pjp.tile([E, S], BF16, tag="pj", name=f"tph{ph_}")
            for si in range(NT):
                nc.tensor.transpose(tph[:, si * P:(si + 1) * P], nrms[si][:],
                                    ident_bf[:])
            nc.vector.tensor_copy(attT_tiles[pm][phs, :], tph[:])
            pend = None

        prev_att = None  # (h, ptiles) whose attended chains interleave next

        for m in range(NM):
            if m + 3 < NM:
                w_pref[m + 3] = prefetch_w(m + 3)
            wqm, wkm = w_pref.pop(m)

            # QT_m / KT_m: [he_pair=128, s=1024], evacuated as bf16
            qkm = []
            for wm, xtiles, lbl in ((wqm, qTq, "qtm"), (wkm, kTq, "ktm")):
                t = qtp.tile([P, S], BF16, tag="qt", name=f"{lbl}{m}")
                for sh in range(2):
                    pst = pjp.tile([P, 512], F32, tag="pj",
                                   name=f"{lbl}ps{m}_{sh}")
                    for j in range(NK):
                        nc.tensor.matmul(
                            pst[:],
                            wm[:, j],
                            xs(xtiles, j)[:, sh * 512:(sh + 1) * 512],
                            start=(j == 0), stop=(j == NK - 1))
                    nc.vector.tensor_copy(t[:, sh * 512:(sh + 1) * 512],
                                          pst[:])
                qkm.append(t)
            qtm, ktm = qkm

            for hh in range(2):
                h = 2 * m + hh
                hs = slice(hh * E, (hh + 1) * E)
                # scoresT + exp -> P_j [t, s] bf16, with the previous head's
                # attended chains interleaved (their exps are already done)
                ptiles = []
                for j in range(NT):
                    pt = ptp.tile([P, S], BF16, tag="pt", name=f"p{h}_{j}")
                    sc = scp.tile([P, S], F32, tag="sc", name=f"sc{h}_{j}")
                    for sh in range(2):
                        nc.tensor.matmul(
                            sc[:, sh * 512:(sh + 1) * 512],
                            ktm[hs, j * P:(j + 1) * P],
                            qtm[hs, sh * 512:(sh + 1) * 512],
                            start=True, stop=True)
                    nc.scalar.activation(pt[:], sc[:], AF.Exp, scale=SCALE)
                    ptiles.append(pt)
                    if prev_att is not None:
                        nrm = emit_att(prev_att[0], prev_att[1], j)
                        prev_att[2].append(nrm)
                if prev_att is not None:
                    ph_, ppt, nrms = prev_att
                    flush_pend()
                    pend = (ph_ // 2, slice((ph_ % 2) * E, (ph_ % 2 + 1) * E),
                            ph_, nrms)
                prev_att = (h, ptiles, [])

        # ---- drain + FC, software-pipelined ------------------------------
        # FC1 (heads of pairs 0..6) runs while the last head's exps drain;
        # its partials (+bias) park in SBUF. The tail is then only the last
        # head's attended, its transposes, and a single-matmul FC2 pass.
        outp = ctx.enter_context(tc.tile_pool(name="outp", bufs=8))
        fc1p = ctx.enter_context(tc.tile_pool(name="fc1p", bufs=2 * NT))
        ph_, ppt, nrms = prev_att
        flush_pend()
        chunks = [(st, oh) for st in range(NT) for oh in range(2)]
        fc1_sb = [None] * len(chunks)

        def emit_fc1(ci):
            st, oh = chunks[ci]
            pso = pjp.tile([P, 512], F32, tag="pj", name=f"fc1_{st}_{oh}")
            for m in range(NM - 1):
                nc.tensor.matmul(
                    pso[:],
                    attT_tiles[m][:, st * P:(st + 1) * P],
                    woT[m][:, oh * 512:(oh + 1) * 512],
                    start=(m == 0), stop=False)
            # rank-1 bias: ones[1,128].T @ bo[1,512] adds bo to every row,
            # and closes the accumulation group
            nc.tensor.matmul(
                pso[:], ones_row[:], bo_bf[:, oh * 512:(oh + 1) * 512],
                start=False, stop=True)
            t = fc1p.tile([P, 512], BF16, tag="fc1", name=f"fc1sb{st}_{oh}")
            # ACT evac: DVE is busy with the last head's normalize chain
            nc.scalar.copy(t[:], pso[:])
            fc1_sb[ci] = t

        def emit_fc2(ci):
            st, oh = chunks[ci]
            # alternate psum pools so the evac never gates the next chunk;
            # the FC1 partial is re-injected on PE via an identity matmul
            pool, tag = (pjp, "pj") if ci % 2 == 0 else (scp, "sc")
            pso = pool.tile([P, 512], F32, tag=tag, name=f"fc2_{st}_{oh}")
            nc.tensor.matmul(
                pso[:],
                attT_tiles[NM - 1][:, st * P:(st + 1) * P],
                woT[NM - 1][:, oh * 512:(oh + 1) * 512],
                start=True, stop=False)
            nc.tensor.matmul(
                pso[:], ident_bf[:], fc1_sb[ci][:], start=False, stop=True)
            ot = outp.tile([P, 512], F32, tag="out", name=f"out{st}_{oh}")
            if ci % 2 == 0:
                nc.vector.tensor_copy(ot[:], pso[:])
            else:
                nc.scalar.copy(ot[:], pso[:])
            nc.sync.dma_start(
                out_d[st * P:(st + 1) * P, oh * 512:(oh + 1) * 512], ot[:])

        # the last head's attended chains go out immediately (their exps
        # finish during the first FC1 chunk), so attT completes early and
        # the serialized out-DMA stream can start near the drain's front
        emit_fc1(0)
        emit_fc1(1)
        for si in range(NT):
            nrms.append(emit_att(ph_, ppt, si))
            if si % 2 == 1 and 2 + si // 2 < len(chunks):
                emit_fc1(2 + si // 2)
        pend = (ph_ // 2, slice((ph_ % 2) * E, (ph_ % 2 + 1) * E), ph_, nrms)
        flush_pend()
        for ci in range(6, len(chunks)):
            emit_fc1(ci)
            emit_fc2(ci - 6)
        for ci in range(len(chunks) - 6, len(chunks)):
            emit_fc2(ci)
    if legalize:
        _legalize_matmul_waits(nc)
    return nc


_NC_CACHE = {}


def _get_nc():
    if "nc" not in _NC_CACHE:
        _NC_CACHE["nc"] = build()
    return _NC_CACHE["nc"]


def kernel(query, key, value, Wq, Wk, Wv, Wo, bo, **run_kwargs):
    query = np.asarray(query, dtype=np.float32)
    key = np.asarray(key, dtype=np.float32)
    value = np.asarray(value, dtype=np.float32)
    Wq = np.ascontiguousarray(np.asarray(Wq, dtype=np.float32))
    Wk = np.ascontiguousarray(np.asarray(Wk, dtype=np.float32))
    Wv = np.ascontiguousarray(np.asarray(Wv, dtype=np.float32))
    Wo = np.ascontiguousarray(np.asarray(Wo, dtype=np.float32))
    bo = np.ascontiguousarray(np.asarray(bo, dtype=np.float32))
    B = query.shape[0]
    assert B == 8, f"expected batch 8, got {B}"

    nc = _get_nc()
    in_maps = []
    for b in range(B):
        in_maps.append({
            "q": np.ascontiguousarray(query[b]),
            "k": np.ascontiguousarray(key[b]),
            "v": np.ascontiguousarray(value[b]),
            "wq": Wq, "wk": Wk, "wv": Wv, "wo": Wo, "bo": bo,
        })
    res = run_bass_kernel_spmd(nc, in_maps, core_ids=list(range(B)),
                               **run_kwargs)
    out = np.stack([r["out"] for r in res.results], axis=0)
    if run_kwargs.get("trace"):
        _NC_CACHE["last_result"] = res
    return out


# revision 5
# speedup vs baseline: 1.0571x; 1.0149x over previous
"""Multi-head attention Trainium2 kernel (nn_MultiHeadAttention_86423331930281).

Self-contained: data-parallel over batch (B=8 -> one batch element per
NeuronCore), runs on cores 0-7 via run_bass_kernel_spmd, returns the full
[8, 1024, 1024] output.

Per-core algorithm (S=1024, D=1024, H=16, E=64), all-bf16 matmul operands:
  - v/q/k: gpsimd cast-load fp32->bf16 two row-blocks per DMA, PE-transpose
    (bf16 identity, 1 cycle/row) into single [128, 8, S] tiles; one-bank
    row-block psum tiles give one evac per row (DVE/ACT alternating)
  - wo: gpsimd cast-load, store to DRAM bf16 scratch, xbar DMA-transpose
    back -> woT [he, out] (entirely off the critical path, SP-issued)
  - wv: gpsimd strided cast-load [ki, ko, h, e]; wq/wk: per-head-pair
    just-in-time gpsimd cast-loads, prefetched 3 pairs ahead
  - V1[t, h, e|1] = vT.T @ Wv with a trailing ones column per head
  - per head-pair: QT/KT [128=2*64, s] = Wq_pair-chunks.T @ qT (8-chunk
    accum in a dedicated psum pool, decoupled from the exp drain)
  - per head: scoresT [t, s] = KT_h-slices.T @ QT_h (K=64), exp on ACT
    (scale=1/32 folded) -> P [t, s] bf16
  - attended in [s, e] orientation with a ONE-HEAD SOFTWARE LAG: the
    previous head's chains att[s, 65] += P[t-chunk, s-chunk].T @ V1 are
    interleaved into the current head's score emission so they never wait
    on the serial exp stream; the 65th column accumulates the softmax
    denominator for free
  - normalize with per-partition reciprocal+multiply (denominator is a
    per-partition scalar in this orientation -- no broadcast round-trip),
    PE re-transpose [s,64]->[64,s] into attT [he, s]
  - FC split: FC1 = attT[m<7].T @ WoT + rank-1 ones.T@bo runs during the
    last head's exp drain, partials parked in SBUF as bf16; FC2 re-injects
    them via an identity matmul on PE, evacs alternate DVE/ACT
"""

import numpy as np
from contextlib import ExitStack

import concourse.bass as bass
import concourse.mybir as mybir
import concourse.tile as tile
from concourse.bass_utils import run_bass_kernel_spmd
from concourse.masks import make_identity

P = 128
S = 1024          # sequence length
DK = 1024         # qkv input dim
H = 16            # heads
E = 64            # per-head dim
HE = H * E        # 1024
OUT = 1024        # output dim
NT = S // P       # 8 s/t tiles
NK = DK // P      # 8 contraction tiles
NM = H // 2       # 8 head pairs
F32 = mybir.dt.float32
BF16 = mybir.dt.bfloat16
AF = mybir.ActivationFunctionType
ALU = mybir.AluOpType
SCALE = 1.0 / 32.0  # 1/sqrt(DK)


def _legalize_matmul_waits(nc):
    """This walrus build allows only ONE sync-wait command per Matmult.
    Move all but the last wait of any multi-wait matmul onto freshly
    inserted PE nops immediately before it — same engine queue, so the
    blocking semantics are identical."""
    SKIP = ("NoOp", "Br", "Halt", "Sem", "Event")
    k = 0
    for f in nc.m.functions:
        for b in f.blocks:
            out = []
            for inst in b.instructions:
                si = getattr(inst, "sync_info", None)
                tname = type(inst).__name__
                if (not any(s in tname for s in SKIP) and si is not None
                        and si.on_wait and len(si.on_wait) > 1):
                    waits = list(si.on_wait)
                    for w in waits[:-1]:
                        nop = mybir.InstNoOp(
                            name=f"legalize-nop-{k}", ins=[], outs=[])
                        k += 1
                        nop.engine = inst.engine
                        nop.sync_info = mybir.SyncInfo(
                            on_wait=[w], on_update=[])
                        out.append(nop)
                    inst.sync_info = mybir.SyncInfo(
                        on_wait=[waits[-1]], on_update=list(si.on_update))
                out.append(inst)
            b.instructions[:] = out
    return k


def build(legalize=True):
    nc = bass.Bass()
    q_d = nc.dram_tensor("q", (S, DK), F32, kind="ExternalInput")
    k_d = nc.dram_tensor("k", (S, DK), F32, kind="ExternalInput")
    v_d = nc.dram_tensor("v", (S, DK), F32, kind="ExternalInput")
    wq_d = nc.dram_tensor("wq", (H, DK, E), F32, kind="ExternalInput")
    wk_d = nc.dram_tensor("wk", (H, DK, E), F32, kind="ExternalInput")
    wv_d = nc.dram_tensor("wv", (H, DK, E), F32, kind="ExternalInput")
    wo_d = nc.dram_tensor("wo", (OUT, HE), F32, kind="ExternalInput")
    bo_d = nc.dram_tensor("bo", (OUT,), F32, kind="ExternalInput")
    out_d = nc.dram_tensor("out", (S, OUT), F32, kind="ExternalOutput")
    wob_d = nc.dram_tensor("wob_scratch", (OUT, HE), BF16, kind="Internal")

    # [h, d, e] viewed as [di, ko, h, e] so partition = inner contraction dim
    wq_v = wq_d.rearrange("h (ko ki) e -> ki ko h e", ki=P)
    wk_v = wk_d.rearrange("h (ko ki) e -> ki ko h e", ki=P)
    wv_v = wv_d.rearrange("h (ko ki) e -> ki ko h e", ki=P)

    with tile.TileContext(nc) as tc, ExitStack() as ctx:
        const = ctx.enter_context(tc.tile_pool(name="const", bufs=1))
        src = ctx.enter_context(tc.tile_pool(name="src", bufs=4))
        xTq = ctx.enter_context(tc.tile_pool(name="xTq", bufs=1))
        woTp = ctx.enter_context(tc.tile_pool(name="woTp", bufs=NK))
        wqkp = ctx.enter_context(tc.tile_pool(name="wqkp", bufs=6))
        v1p = ctx.enter_context(tc.tile_pool(name="v1p", bufs=NT))
        # scores psum: 2 x 2 banks; proj/fc psum: 2 x 1 bank (decoupled so
        # projections never wait on the exp drain tail); the re-transpose
        # collect tiles share the proj slots (same tag, same bank size)
        scp = ctx.enter_context(tc.tile_pool(name="scp", bufs=2, space="PSUM"))
        pjp = ctx.enter_context(tc.tile_pool(name="pjp", bufs=2, space="PSUM"))
        ph1 = ExitStack()
        vTp = ph1.enter_context(tc.tile_pool(name="vTp", bufs=1))
        wvp = ph1.enter_context(tc.tile_pool(name="wvp", bufs=2))


        # ---- load + transpose phase --------------------------------------
        # one [128, NK, S] tile per transposed matrix; chunk j = [:, j, :]
        vTq = vTp.tile([P, NK, S], BF16, name="vT", tag="vT")
        qTq = xTq.tile([P, NK, S], BF16, name="qT", tag="qT")
        kTq = xTq.tile([P, NK, S], BF16, name="kT", tag="kT")
        woT = [woTp.tile([P, S], BF16, name=f"woT{j}", tag="woT")
               for j in range(NK)]

        tpp = ph1.enter_context(tc.tile_pool(name="tpp", bufs=2,
                                             space="PSUM"))

        NB = 2  # row-blocks per cast DMA (NB=4 halves the Pool dge count
        #         but crashes NRT execution; NB=2 is hardware-verified)

        def emit_casts(mat_d, nm, n=None):
            """cast-load NB row-blocks per DMA"""
            stbs = []
            for rr in range(n if n is not None else NT // NB):
                stb = src.tile([P, NB, DK], BF16, tag="srcb",
                               name=f"{nm}cast{rr}")
                nc.gpsimd.dma_start(
                    stb[:],
                    mat_d[rr * NB * P:(rr + 1) * NB * P, :].rearrange(
                        "(c p) d -> p c d", c=NB))
                stbs.append(stb)
            return stbs

        def emit_transposes(stbs, tile, nm):
            """PE-transpose a full row-block into one 1-bank psum tile;
            single evac per row-block (DVE/ACT alternating — GPSIMD cannot
            read PSUM)"""
            for r in range(NT):
                stb = stbs[r // NB]
                c = r % NB
                # alternate with the (still idle) scores pool slots so the
                # evac WAR never paces the transposes
                pool, tg = (tpp, "tp") if r % 2 == 0 else (scp, "sc")
                pt_ = pool.tile([P, NK, P], BF16, tag=tg, name=f"{nm}ps{r}")
                for j in range(NK):
                    nc.tensor.transpose(
                        pt_[:, j, :], stb[:, c, j * P:(j + 1) * P],
                        ident_bf[:])
                if r % 2 == 0:
                    nc.vector.tensor_copy(
                        tile[:, :, r * P:(r + 1) * P], pt_[:])
                else:
                    nc.scalar.copy(
                        tile[:, :, r * P:(r + 1) * P], pt_[:])

        def pe_transpose(mat_d, tile, nm):
            emit_transposes(emit_casts(mat_d, nm), tile, nm)

        def xs(tile, j):
            """[128, S] view of transposed chunk j"""
            return tile[:, j, :]

        # first v cast goes out before the identity init so data and
        # identity land together for the first transpose
        v_stbs = emit_casts(v_d[0:NB * P, :], "v0", n=1)
        ident = const.tile([P, P], F32, name="ident")
        make_identity(nc, ident)
        ident_bf = const.tile([P, P], BF16, name="ident_bf")
        nc.vector.tensor_copy(ident_bf[:], ident[:])
        v_stbs += emit_casts(v_d[NB * P:, :], "v1", n=3)
        emit_transposes(v_stbs, vTq, "v")

        # wv: strided cast-load [ki, ko, h, e] per contraction chunk
        wv_sb = []
        for half in range(2):
            t = wvp.tile([P, NK // 2, H, E], BF16, tag="wwv",
                         name=f"wvsb{half}")
            for jj in range(NK // 2):
                nc.gpsimd.dma_start(
                    t[:, jj], wv_v[:, half * (NK // 2) + jj])
            wv_sb.append(t)

        def prefetch_w(m):
            """per-pair just-in-time Wq/Wk chunk loads [ki, ko, 2, e]"""
            wqm = wqkp.tile([P, NK, 2, E], BF16, tag="wqk", name=f"wqm{m}")
            wkm = wqkp.tile([P, NK, 2, E], BF16, tag="wqk", name=f"wkm{m}")
            for hh in range(2):
                nc.gpsimd.dma_start(wqm[:, :, hh, :], wq_v[:, :, 2 * m + hh, :])
                nc.gpsimd.dma_start(wkm[:, :, hh, :], wk_v[:, :, 2 * m + hh, :])
            return wqm, wkm

        w_pref = {0: prefetch_w(0)}

        # ---- V projection: V1 [t, h, e|ones] ------------------------------
        v1_tiles = []
        for i in range(NT):
            v1 = v1p.tile([P, H, E + 1], BF16, tag="v1", name=f"v1_{i}")
            nc.gpsimd.memset(v1[:, :, E], 1.0)
            for nh in range(2):
                pst = pjp.tile([P, 512], F32, tag="pj", name=f"vproj{i}_{nh}")
                for j in range(NK):
                    wvf = wv_sb[j // (NK // 2)][:, j % (NK // 2)].rearrange(
                        "p h e -> p (h e)")
                    nc.tensor.matmul(
                        pst[:],
                        xs(vTq, j)[:, i * P:(i + 1) * P],
                        wvf[:, nh * 512:(nh + 1) * 512],
                        start=(j == 0), stop=(j == NK - 1))
                nc.vector.tensor_copy(
                    v1[:, nh * (H // 2):(nh + 1) * (H // 2), 0:E],
                    pst[:].rearrange("p (h e) -> p h e", e=E))
            v1_tiles.append(v1)

        w_pref[1] = prefetch_w(1)
        pe_transpose(q_d, qTq, "q")
        w_pref[2] = prefetch_w(2)
        pe_transpose(k_d, kTq, "k")

        # FC-only constants, emitted after the critical-path loads
        ones_row = const.tile([1, P], BF16, name="ones_row")
        nc.gpsimd.memset(ones_row[:], 1.0)
        bo_bf = const.tile([1, OUT], BF16, name="bo_bf")
        nc.gpsimd.dma_start(bo_bf[:], bo_d[None, :])

        ph1.close()

        # ---- attention (one-head software pipeline lag) -------------------
        qtp = ctx.enter_context(tc.tile_pool(name="qtp", bufs=4))
        ptp = ctx.enter_context(tc.tile_pool(name="ptp", bufs=2 * NT))
        normp = ctx.enter_context(tc.tile_pool(name="normp", bufs=16))
        denp = ctx.enter_context(tc.tile_pool(name="denp", bufs=8))
        attp = ctx.enter_context(tc.tile_pool(name="attp", bufs=NM))
        att_ps = ctx.enter_context(
            tc.tile_pool(name="att_ps", bufs=2, space="PSUM"))

        attT_tiles = [attp.tile([P, S], BF16, tag="attT", name=f"attT{m}")
                      for m in range(NM)]

        # wo: cast-load bf16 (gpsimd, queued after the critical-path loads),
        # store to scratch (SP), xbar DMA-transpose back (SP) — SP is
        # otherwise idle until the out writes; needed only by the FC
        for rr in range(NT // 2):
            stb = src.tile([P, 2, DK], BF16, tag="srcb", name=f"wocast{rr}")
            nc.gpsimd.dma_start(
                stb[:],
                wo_d[rr * 2 * P:(rr + 1) * 2 * P, :].rearrange(
                    "(c p) d -> p c d", c=2))
            for c in range(2):
                r = rr * 2 + c
                nc.sync.dma_start(wob_d[r * P:(r + 1) * P, :], stb[:, c, :])
        for j in range(NK):
            nc.sync.dma_start_transpose(
                woT[j][:], wob_d[:, j * P:(j + 1) * P])

        def emit_att(h, ptiles, si):
            """attended [s-chunk si, e|denom] for head h + normalize"""
            if si % 2 == 0:
                _att_slot[0] = att_ps.tile([P, 2, E + 1], F32, tag="attps",
                                           name=f"att{h}_{si}")
            aps = _att_slot[0][:, si % 2, :]
            for j in range(NT):
                nc.tensor.matmul(
                    aps[0:P, 0:E + 1],
                    ptiles[j][:, si * P:(si + 1) * P],
                    v1_tiles[j][:, h, :],
                    start=(j == 0), stop=(j == NT - 1))
            den = denp.tile([P, 1], F32, tag="den", name=f"den{h}_{si}")
            nc.vector.reciprocal(den[:], aps[0:P, E:E + 1])
            nrm = normp.tile([P, E], BF16, tag="nrm", name=f"nrm{h}_{si}")
            nc.vector.tensor_scalar(nrm[:], aps[0:P, 0:E], den[:], None,
                                    ALU.mult)
            return nrm

        _att_slot = [None]
        pend = None  # (m, hs, nrm list) awaiting re-transpose + evac

        def flush_pend():
            nonlocal pend
            if pend is None:
                return
            pm, phs, ph_, nrms = pend
            tph = pjp.tile([E, S], BF16, tag="pj", name=f"tph{ph_}")
            for si in range(NT):
                nc.tensor.transpose(tph[:, si * P:(si + 1) * P], nrms[si][:],
                                    ident_bf[:])
            nc.vector.tensor_copy(attT_tiles[pm][phs, :], tph[:])
            pend = None

        prev_att = None  # (h, ptiles) whose attended chains interleave next

        for m in range(NM):
            if m + 3 < NM:
                w_pref[m + 3] = prefetch_w(m + 3)
            wqm, wkm = w_pref.pop(m)

            # QT_m / KT_m: [he_pair=128, s=1024], evacuated as bf16
            qkm = []
            for wm, xtiles, lbl in ((wqm, qTq, "qtm"), (wkm, kTq, "ktm")):
                t = qtp.tile([P, S], BF16, tag="qt", name=f"{lbl}{m}")
                for sh in range(2):
                    pst = pjp.tile([P, 512], F32, tag="pj",
                                   name=f"{lbl}ps{m}_{sh}")
                    for j in range(NK):
                        nc.tensor.matmul(
                            pst[:],
                            wm[:, j],
                            xs(xtiles, j)[:, sh * 512:(sh + 1) * 512],
                            start=(j == 0), stop=(j == NK - 1))
                    nc.vector.tensor_copy(t[:, sh * 512:(sh + 1) * 512],
                                          pst[:])
                qkm.append(t)
            qtm, ktm = qkm

            for hh in range(2):
                h = 2 * m + hh
                hs = slice(hh * E, (hh + 1) * E)
                # scoresT + exp -> P_j [t, s] bf16, with the previous head's
                # attended chains interleaved (their exps are already done)
                ptiles = []
                for j in range(NT):
                    pt = ptp.tile([P, S], BF16, tag="pt", name=f"p{h}_{j}")
                    sc = scp.tile([P, S], F32, tag="sc", name=f"sc{h}_{j}")
                    for sh in range(2):
                        nc.tensor.matmul(
                            sc[:, sh * 512:(sh + 1) * 512],
                            ktm[hs, j * P:(j + 1) * P],
                            qtm[hs, sh * 512:(sh + 1) * 512],
                            start=True, stop=True)
                    nc.scalar.activation(pt[:], sc[:], AF.Exp, scale=SCALE)
                    ptiles.append(pt)
                    if prev_att is not None:
                        nrm = emit_att(prev_att[0], prev_att[1], j)
                        prev_att[2].append(nrm)
                if prev_att is not None:
                    ph_, ppt, nrms = prev_att
                    flush_pend()
                    pend = (ph_ // 2, slice((ph_ % 2) * E, (ph_ % 2 + 1) * E),
                            ph_, nrms)
                prev_att = (h, ptiles, [])

        # ---- drain + FC, software-pipelined ------------------------------
        # FC1 (heads of pairs 0..6) runs while the last head's exps drain;
        # its partials (+bias) park in SBUF. The tail is then only the last
        # head's attended, its transposes, and a single-matmul FC2 pass.
        outp = ctx.enter_context(tc.tile_pool(name="outp", bufs=8))
        fc1p = ctx.enter_context(tc.tile_pool(name="fc1p", bufs=2 * NT))
        ph_, ppt, nrms = prev_att
        flush_pend()
        chunks = [(st, oh) for st in range(NT) for oh in range(2)]
        fc1_sb = [None] * len(chunks)

        def emit_fc1(ci):
            st, oh = chunks[ci]
            pso = pjp.tile([P, 512], F32, tag="pj", name=f"fc1_{st}_{oh}")
            for m in range(NM - 1):
                nc.tensor.matmul(
                    pso[:],
                    attT_tiles[m][:, st * P:(st + 1) * P],
                    woT[m][:, oh * 512:(oh + 1) * 512],
                    start=(m == 0), stop=False)
            # rank-1 bias: ones[1,128].T @ bo[1,512] adds bo to every row,
            # and closes the accumulation group
            nc.tensor.matmul(
                pso[:], ones_row[:], bo_bf[:, oh * 512:(oh + 1) * 512],
                start=False, stop=True)
            t = fc1p.tile([P, 512], BF16, tag="fc1", name=f"fc1sb{st}_{oh}")
            # ACT evac: DVE is busy with the last head's normalize chain
            nc.scalar.copy(t[:], pso[:])
            fc1_sb[ci] = t

        def emit_fc2(ci):
            st, oh = chunks[ci]
            # alternate psum pools so the evac never gates the next chunk;
            # the FC1 partial is re-injected on PE via an identity matmul
            pool, tag = (pjp, "pj") if ci % 2 == 0 else (scp, "sc")
            pso = pool.tile([P, 512], F32, tag=tag, name=f"fc2_{st}_{oh}")
            nc.tensor.matmul(
                pso[:],
                attT_tiles[NM - 1][:, st * P:(st + 1) * P],
                woT[NM - 1][:, oh * 512:(oh + 1) * 512],
                start=True, stop=False)
            nc.tensor.matmul(
                pso[:], ident_bf[:], fc1_sb[ci][:], start=False, stop=True)
            ot = outp.tile([P, 512], F32, tag="out", name=f"out{st}_{oh}")
            if ci % 2 == 0:
                nc.vector.tensor_copy(ot[:], pso[:])
            else:
                nc.scalar.copy(ot[:], pso[:])
            nc.sync.dma_start(
                out_d[st * P:(st + 1) * P, oh * 512:(oh + 1) * 512], ot[:])

        # the last head's attended chains go out immediately (their exps
        # finish during the first FC1 chunk), so attT completes early and
        # the serialized out-DMA stream can start near the drain's front
        emit_fc1(0)
        emit_fc1(1)
        for si in range(NT):
            nrms.append(emit_att(ph_, ppt, si))
            if si % 2 == 1 and 2 + si // 2 < len(chunks):
                emit_fc1(2 + si // 2)
        pend = (ph_ // 2, slice((ph_ % 2) * E, (ph_ % 2 + 1) * E), ph_, nrms)
        flush_pend()
        for ci in range(6, len(chunks)):
            emit_fc1(ci)
            emit_fc2(ci - 6)
        for ci in range(len(chunks) - 6, len(chunks)):
            emit_fc2(ci)
    if legalize:
        _legalize_matmul_waits(nc)
    return nc


_NC_CACHE = {}


def _get_nc():
    if "nc" not in _NC_CACHE:
        _NC_CACHE["nc"] = build()
    return _NC_CACHE["nc"]


def kernel(query, key, value, Wq, Wk, Wv, Wo, bo, **run_kwargs):
    query = np.asarray(query, dtype=np.float32)
    key = np.asarray(key, dtype=np.float32)
    value = np.asarray(value, dtype=np.float32)
    Wq = np.ascontiguousarray(np.asarray(Wq, dtype=np.float32))
    Wk = np.ascontiguousarray(np.asarray(Wk, dtype=np.float32))
    Wv = np.ascontiguousarray(np.asarray(Wv, dtype=np.float32))
    Wo = np.ascontiguousarray(np.asarray(Wo, dtype=np.float32))
    bo = np.ascontiguousarray(np.asarray(bo, dtype=np.float32))
    B = query.shape[0]
    assert B == 8, f"expected batch 8, got {B}"

    nc = _get_nc()
    in_maps = []
    for b in range(B):
        in_maps.append({
            "q": np.ascontiguousarray(query[b]),
            "k": np.ascontiguousarray(key[b]),
            "v": np.ascontiguousarray(value[b]),
            "wq": Wq, "wk": Wk, "wv": Wv, "wo": Wo, "bo": bo,
        })
    res = run_bass_kernel_spmd(nc, in_maps, core_ids=list(range(B)),
                               **run_kwargs)
    out = np.stack([r["out"] for r in res.results], axis=0)
    if run_kwargs.get("trace"):
        _NC_CACHE["last_result"] = res
    return out


# revision 6
# speedup vs baseline: 1.0663x; 1.0087x over previous
"""Multi-head attention Trainium2 kernel (nn_MultiHeadAttention_86423331930281).

Self-contained: data-parallel over batch (B=8 -> one batch element per
NeuronCore), runs on cores 0-7 via run_bass_kernel_spmd, returns the full
[8, 1024, 1024] output.

Per-core algorithm (S=1024, D=1024, H=16, E=64), all-bf16 matmul operands:
  - v/q/k: gpsimd cast-load fp32->bf16 two row-blocks per DMA, PE-transpose
    (bf16 identity, 1 cycle/row) into single [128, 8, S] tiles; one-bank
    row-block psum tiles give one evac per row (DVE/ACT alternating)
  - wo: gpsimd cast-load, store to DRAM bf16 scratch, xbar DMA-transpose
    back -> woT [he, out] (entirely off the critical path, SP-issued)
  - wv: gpsimd strided cast-load [ki, ko, h, e]; wq/wk: per-head-pair
    just-in-time gpsimd cast-loads, prefetched 3 pairs ahead
  - V1[t, h, e|1] = vT.T @ Wv with a trailing ones column per head
  - per head-pair: QT/KT [128=2*64, s] = Wq_pair-chunks.T @ qT (8-chunk
    accum in a dedicated psum pool, decoupled from the exp drain)
  - per head: scoresT [t, s] = KT_h-slices.T @ QT_h (K=64), exp on ACT
    (scale=1/32 folded) -> P [t, s] bf16
  - attended in [s, e] orientation with a ONE-HEAD SOFTWARE LAG: the
    previous head's chains att[s, 65] += P[t-chunk, s-chunk].T @ V1 are
    interleaved into the current head's score emission so they never wait
    on the serial exp stream; the 65th column accumulates the softmax
    denominator for free
  - normalize with per-partition reciprocal+multiply (denominator is a
    per-partition scalar in this orientation -- no broadcast round-trip),
    PE re-transpose [s,64]->[64,s] into attT [he, s]
  - FC split: FC1 = attT[m<7].T @ WoT runs during the last head's exp
    drain; partials (+bias, f32 DVE add) park in SBUF as bf16 and FC2's
    DVE evac adds them to the m=7 contribution -- no extra PE matmuls
"""

import numpy as np
from contextlib import ExitStack

import concourse.bass as bass
import concourse.mybir as mybir
import concourse.tile as tile
from concourse.bass_utils import run_bass_kernel_spmd
from concourse.masks import make_identity

P = 128
S = 1024          # sequence length
DK = 1024         # qkv input dim
H = 16            # heads
E = 64            # per-head dim
HE = H * E        # 1024
OUT = 1024        # output dim
NT = S // P       # 8 s/t tiles
NK = DK // P      # 8 contraction tiles
NM = H // 2       # 8 head pairs
F32 = mybir.dt.float32
BF16 = mybir.dt.bfloat16
AF = mybir.ActivationFunctionType
ALU = mybir.AluOpType
SCALE = 1.0 / 32.0  # 1/sqrt(DK)


def _legalize_matmul_waits(nc):
    """This walrus build allows only ONE sync-wait command per Matmult.
    Move all but the last wait of any multi-wait matmul onto freshly
    inserted PE nops immediately before it — same engine queue, so the
    blocking semantics are identical."""
    SKIP = ("NoOp", "Br", "Halt", "Sem", "Event")
    k = 0
    for f in nc.m.functions:
        for b in f.blocks:
            out = []
            for inst in b.instructions:
                si = getattr(inst, "sync_info", None)
                tname = type(inst).__name__
                if (not any(s in tname for s in SKIP) and si is not None
                        and si.on_wait and len(si.on_wait) > 1):
                    waits = list(si.on_wait)
                    for w in waits[:-1]:
                        nop = mybir.InstNoOp(
                            name=f"legalize-nop-{k}", ins=[], outs=[])
                        k += 1
                        nop.engine = inst.engine
                        nop.sync_info = mybir.SyncInfo(
                            on_wait=[w], on_update=[])
                        out.append(nop)
                    inst.sync_info = mybir.SyncInfo(
                        on_wait=[waits[-1]], on_update=list(si.on_update))
                out.append(inst)
            b.instructions[:] = out
    return k


def build(legalize=True):
    nc = bass.Bass()
    q_d = nc.dram_tensor("q", (S, DK), F32, kind="ExternalInput")
    k_d = nc.dram_tensor("k", (S, DK), F32, kind="ExternalInput")
    v_d = nc.dram_tensor("v", (S, DK), F32, kind="ExternalInput")
    wq_d = nc.dram_tensor("wq", (H, DK, E), F32, kind="ExternalInput")
    wk_d = nc.dram_tensor("wk", (H, DK, E), F32, kind="ExternalInput")
    wv_d = nc.dram_tensor("wv", (H, DK, E), F32, kind="ExternalInput")
    wo_d = nc.dram_tensor("wo", (OUT, HE), F32, kind="ExternalInput")
    bo_d = nc.dram_tensor("bo", (OUT,), F32, kind="ExternalInput")
    out_d = nc.dram_tensor("out", (S, OUT), F32, kind="ExternalOutput")
    wob_d = nc.dram_tensor("wob_scratch", (OUT, HE), BF16, kind="Internal")

    # [h, d, e] viewed as [di, ko, h, e] so partition = inner contraction dim
    wq_v = wq_d.rearrange("h (ko ki) e -> ki ko h e", ki=P)
    wk_v = wk_d.rearrange("h (ko ki) e -> ki ko h e", ki=P)
    wv_v = wv_d.rearrange("h (ko ki) e -> ki ko h e", ki=P)

    with tile.TileContext(nc) as tc, ExitStack() as ctx:
        const = ctx.enter_context(tc.tile_pool(name="const", bufs=1))
        src = ctx.enter_context(tc.tile_pool(name="src", bufs=4))
        xTq = ctx.enter_context(tc.tile_pool(name="xTq", bufs=1))
        woTp = ctx.enter_context(tc.tile_pool(name="woTp", bufs=NK))
        wqkp = ctx.enter_context(tc.tile_pool(name="wqkp", bufs=6))
        v1p = ctx.enter_context(tc.tile_pool(name="v1p", bufs=NT))
        # scores psum: 2 x 2 banks; proj/fc psum: 2 x 1 bank (decoupled so
        # projections never wait on the exp drain tail); the re-transpose
        # collect tiles share the proj slots (same tag, same bank size)
        scp = ctx.enter_context(tc.tile_pool(name="scp", bufs=2, space="PSUM"))
        pjp = ctx.enter_context(tc.tile_pool(name="pjp", bufs=2, space="PSUM"))
        ph1 = ExitStack()
        vTp = ph1.enter_context(tc.tile_pool(name="vTp", bufs=1))
        wvp = ph1.enter_context(tc.tile_pool(name="wvp", bufs=2))


        # ---- load + transpose phase --------------------------------------
        # one [128, NK, S] tile per transposed matrix; chunk j = [:, j, :]
        vTq = vTp.tile([P, NK, S], BF16, name="vT", tag="vT")
        qTq = xTq.tile([P, NK, S], BF16, name="qT", tag="qT")
        kTq = xTq.tile([P, NK, S], BF16, name="kT", tag="kT")
        woT = [woTp.tile([P, S], BF16, name=f"woT{j}", tag="woT")
               for j in range(NK)]

        tpp = ph1.enter_context(tc.tile_pool(name="tpp", bufs=2,
                                             space="PSUM"))

        NB = 2  # row-blocks per cast DMA (NB=4 halves the Pool dge count
        #         but crashes NRT execution; NB=2 is hardware-verified)

        def emit_casts(mat_d, nm, n=None):
            """cast-load NB row-blocks per DMA"""
            stbs = []
            for rr in range(n if n is not None else NT // NB):
                stb = src.tile([P, NB, DK], BF16, tag="srcb",
                               name=f"{nm}cast{rr}")
                nc.gpsimd.dma_start(
                    stb[:],
                    mat_d[rr * NB * P:(rr + 1) * NB * P, :].rearrange(
                        "(c p) d -> p c d", c=NB))
                stbs.append(stb)
            return stbs

        def emit_transposes(stbs, tile, nm):
            """PE-transpose a full row-block into one 1-bank psum tile;
            single evac per row-block (DVE/ACT alternating — GPSIMD cannot
            read PSUM)"""
            for r in range(NT):
                stb = stbs[r // NB]
                c = r % NB
                # alternate with the (still idle) scores pool slots so the
                # evac WAR never paces the transposes
                pool, tg = (tpp, "tp") if r % 2 == 0 else (scp, "sc")
                pt_ = pool.tile([P, NK, P], BF16, tag=tg, name=f"{nm}ps{r}")
                for j in range(NK):
                    nc.tensor.transpose(
                        pt_[:, j, :], stb[:, c, j * P:(j + 1) * P],
                        ident_bf[:])
                if r % 2 == 0:
                    nc.vector.tensor_copy(
                        tile[:, :, r * P:(r + 1) * P], pt_[:])
                else:
                    nc.scalar.copy(
                        tile[:, :, r * P:(r + 1) * P], pt_[:])

        def pe_transpose(mat_d, tile, nm):
            emit_transposes(emit_casts(mat_d, nm), tile, nm)

        def xs(tile, j):
            """[128, S] view of transposed chunk j"""
            return tile[:, j, :]

        # first v cast goes out before the identity init so data and
        # identity land together for the first transpose
        v_stbs = emit_casts(v_d[0:NB * P, :], "v0", n=1)
        ident = const.tile([P, P], F32, name="ident")
        make_identity(nc, ident)
        ident_bf = const.tile([P, P], BF16, name="ident_bf")
        nc.vector.tensor_copy(ident_bf[:], ident[:])
        v_stbs += emit_casts(v_d[NB * P:, :], "v1", n=3)
        emit_transposes(v_stbs, vTq, "v")

        # wv: strided cast-load [ki, ko, h, e] per contraction chunk
        wv_sb = []
        for half in range(2):
            t = wvp.tile([P, NK // 2, H, E], BF16, tag="wwv",
                         name=f"wvsb{half}")
            for jj in range(NK // 2):
                nc.gpsimd.dma_start(
                    t[:, jj], wv_v[:, half * (NK // 2) + jj])
            wv_sb.append(t)

        def prefetch_w(m):
            """per-pair just-in-time Wq/Wk chunk loads [ki, ko, 2, e]"""
            wqm = wqkp.tile([P, NK, 2, E], BF16, tag="wqk", name=f"wqm{m}")
            wkm = wqkp.tile([P, NK, 2, E], BF16, tag="wqk", name=f"wkm{m}")
            for hh in range(2):
                nc.gpsimd.dma_start(wqm[:, :, hh, :], wq_v[:, :, 2 * m + hh, :])
                nc.gpsimd.dma_start(wkm[:, :, hh, :], wk_v[:, :, 2 * m + hh, :])
            return wqm, wkm

        w_pref = {0: prefetch_w(0)}

        # ---- V projection: V1 [t, h, e|ones] ------------------------------
        v1_tiles = []
        for i in range(NT):
            v1 = v1p.tile([P, H, E + 1], BF16, tag="v1", name=f"v1_{i}")
            nc.gpsimd.memset(v1[:, :, E], 1.0)
            for nh in range(2):
                pst = pjp.tile([P, 512], F32, tag="pj", name=f"vproj{i}_{nh}")
                for j in range(NK):
                    wvf = wv_sb[j // (NK // 2)][:, j % (NK // 2)].rearrange(
                        "p h e -> p (h e)")
                    nc.tensor.matmul(
                        pst[:],
                        xs(vTq, j)[:, i * P:(i + 1) * P],
                        wvf[:, nh * 512:(nh + 1) * 512],
                        start=(j == 0), stop=(j == NK - 1))
                nc.vector.tensor_copy(
                    v1[:, nh * (H // 2):(nh + 1) * (H // 2), 0:E],
                    pst[:].rearrange("p (h e) -> p h e", e=E))
            v1_tiles.append(v1)

        w_pref[1] = prefetch_w(1)
        pe_transpose(q_d, qTq, "q")
        w_pref[2] = prefetch_w(2)
        pe_transpose(k_d, kTq, "k")

        # FC-only constant, emitted after the critical-path loads (SP HWDGE)
        bo_bc = const.tile([P, OUT], F32, name="bo_bc")
        nc.sync.dma_start(bo_bc[:], bo_d[None, :].to_broadcast((P, OUT)))

        ph1.close()

        # ---- attention (one-head software pipeline lag) -------------------
        qtp = ctx.enter_context(tc.tile_pool(name="qtp", bufs=4))
        ptp = ctx.enter_context(tc.tile_pool(name="ptp", bufs=2 * NT))
        normp = ctx.enter_context(tc.tile_pool(name="normp", bufs=16))
        denp = ctx.enter_context(tc.tile_pool(name="denp", bufs=8))
        attp = ctx.enter_context(tc.tile_pool(name="attp", bufs=NM))
        att_ps = ctx.enter_context(
            tc.tile_pool(name="att_ps", bufs=2, space="PSUM"))

        attT_tiles = [attp.tile([P, S], BF16, tag="attT", name=f"attT{m}")
                      for m in range(NM)]

        # wo: cast-load bf16 (gpsimd, queued after the critical-path loads),
        # store to scratch (SP), xbar DMA-transpose back (SP) — SP is
        # otherwise idle until the out writes; needed only by the FC
        for rr in range(NT // 2):
            stb = src.tile([P, 2, DK], BF16, tag="srcb", name=f"wocast{rr}")
            nc.gpsimd.dma_start(
                stb[:],
                wo_d[rr * 2 * P:(rr + 1) * 2 * P, :].rearrange(
                    "(c p) d -> p c d", c=2))
            for c in range(2):
                r = rr * 2 + c
                nc.sync.dma_start(wob_d[r * P:(r + 1) * P, :], stb[:, c, :])
        for j in range(NK):
            nc.sync.dma_start_transpose(
                woT[j][:], wob_d[:, j * P:(j + 1) * P])

        def emit_att(h, ptiles, si):
            """attended [s-chunk si, e|denom] for head h + normalize"""
            if si % 2 == 0:
                _att_slot[0] = att_ps.tile([P, 2, E + 1], F32, tag="attps",
                                           name=f"att{h}_{si}")
            aps = _att_slot[0][:, si % 2, :]
            for j in range(NT):
                nc.tensor.matmul(
                    aps[0:P, 0:E + 1],
                    ptiles[j][:, si * P:(si + 1) * P],
                    v1_tiles[j][:, h, :],
                    start=(j == 0), stop=(j == NT - 1))
            den = denp.tile([P, 1], F32, tag="den", name=f"den{h}_{si}")
            nc.vector.reciprocal(den[:], aps[0:P, E:E + 1])
            nrm = normp.tile([P, E], BF16, tag="nrm", name=f"nrm{h}_{si}")
            nc.vector.tensor_scalar(nrm[:], aps[0:P, 0:E], den[:], None,
                                    ALU.mult)
            return nrm

        _att_slot = [None]
        pend = None  # (m, hs, nrm list) awaiting re-transpose + evac

        def flush_pend():
            nonlocal pend
            if pend is None:
                return
            pm, phs, ph_, nrms = pend
            tph = pjp.tile([E, S], BF16, tag="pj", name=f"tph{ph_}")
            for si in range(NT):
                nc.tensor.transpose(tph[:, si * P:(si + 1) * P], nrms[si][:],
                                    ident_bf[:])
            nc.vector.tensor_copy(attT_tiles[pm][phs, :], tph[:])
            pend = None

        prev_att = None  # (h, ptiles) whose attended chains interleave next

        for m in range(NM):
            if m + 3 < NM:
                w_pref[m + 3] = prefetch_w(m + 3)
            wqm, wkm = w_pref.pop(m)

            # QT_m / KT_m: [he_pair=128, s=1024], evacuated as bf16
            qkm = []
            for wm, xtiles, lbl in ((wqm, qTq, "qtm"), (wkm, kTq, "ktm")):
                t = qtp.tile([P, S], BF16, tag="qt", name=f"{lbl}{m}")
                for sh in range(2):
                    pst = pjp.tile([P, 512], F32, tag="pj",
                                   name=f"{lbl}ps{m}_{sh}")
                    for j in range(NK):
                        nc.tensor.matmul(
                            pst[:],
                            wm[:, j],
                            xs(xtiles, j)[:, sh * 512:(sh + 1) * 512],
                            start=(j == 0), stop=(j == NK - 1))
                    nc.vector.tensor_copy(t[:, sh * 512:(sh + 1) * 512],
                                          pst[:])
                qkm.append(t)
            qtm, ktm = qkm

            for hh in range(2):
                h = 2 * m + hh
                hs = slice(hh * E, (hh + 1) * E)
                # scoresT + exp -> P_j [t, s] bf16, with the previous head's
                # attended chains interleaved (their exps are already done)
                ptiles = []
                for j in range(NT):
                    pt = ptp.tile([P, S], BF16, tag="pt", name=f"p{h}_{j}")
                    sc = scp.tile([P, S], F32, tag="sc", name=f"sc{h}_{j}")
                    for sh in range(2):
                        nc.tensor.matmul(
                            sc[:, sh * 512:(sh + 1) * 512],
                            ktm[hs, j * P:(j + 1) * P],
                            qtm[hs, sh * 512:(sh + 1) * 512],
                            start=True, stop=True)
                    nc.scalar.activation(pt[:], sc[:], AF.Exp, scale=SCALE)
                    ptiles.append(pt)
                    if prev_att is not None:
                        nrm = emit_att(prev_att[0], prev_att[1], j)
                        prev_att[2].append(nrm)
                if prev_att is not None:
                    ph_, ppt, nrms = prev_att
                    flush_pend()
                    pend = (ph_ // 2, slice((ph_ % 2) * E, (ph_ % 2 + 1) * E),
                            ph_, nrms)
                prev_att = (h, ptiles, [])

        # ---- drain + FC, software-pipelined ------------------------------
        # FC1 (heads of pairs 0..6) runs while the last head's exps drain;
        # its partials (+bias) park in SBUF. The tail is then only the last
        # head's attended, its transposes, and a single-matmul FC2 pass.
        outp = ctx.enter_context(tc.tile_pool(name="outp", bufs=8))
        fc1p = ctx.enter_context(tc.tile_pool(name="fc1p", bufs=2 * NT))
        ph_, ppt, nrms = prev_att
        flush_pend()
        chunks = [(st, oh) for st in range(NT) for oh in range(2)]
        fc1_sb = [None] * len(chunks)

        def emit_fc1(ci):
            st, oh = chunks[ci]
            pso = pjp.tile([P, 512], F32, tag="pj", name=f"fc1_{st}_{oh}")
            for m in range(NM - 1):
                nc.tensor.matmul(
                    pso[:],
                    attT_tiles[m][:, st * P:(st + 1) * P],
                    woT[m][:, oh * 512:(oh + 1) * 512],
                    start=(m == 0), stop=(m == NM - 2))
            t = fc1p.tile([P, 512], BF16, tag="fc1", name=f"fc1sb{st}_{oh}")
            # DVE evac folds the bias in; the last head's normalize chain is
            # already drained by the early att emission, so no hostage stall
            nc.vector.tensor_tensor(
                t[:], pso[:], bo_bc[:, oh * 512:(oh + 1) * 512], ALU.add)
            fc1_sb[ci] = t

        def emit_fc2(ci):
            st, oh = chunks[ci]
            # alternate psum pools so the evac never gates the next chunk;
            # the FC1 partial is re-injected on PE via an identity matmul
            pool, tag = (pjp, "pj") if ci % 2 == 0 else (scp, "sc")
            pso = pool.tile([P, 512], F32, tag=tag, name=f"fc2_{st}_{oh}")
            nc.tensor.matmul(
                pso[:],
                attT_tiles[NM - 1][:, st * P:(st + 1) * P],
                woT[NM - 1][:, oh * 512:(oh + 1) * 512],
                start=True, stop=True)
            ot = outp.tile([P, 512], F32, tag="out", name=f"out{st}_{oh}")
            nc.vector.tensor_tensor(ot[:], pso[:], fc1_sb[ci][:], ALU.add)
            nc.sync.dma_start(
                out_d[st * P:(st + 1) * P, oh * 512:(oh + 1) * 512], ot[:])

        # the last head's attended chains go out immediately (their exps
        # finish during the first FC1 chunk), so attT completes early and
        # the serialized out-DMA stream can start near the drain's front
        emit_fc1(0)
        emit_fc1(1)
        for si in range(NT):
            nrms.append(emit_att(ph_, ppt, si))
            if si % 2 == 1 and 2 + si // 2 < len(chunks):
                emit_fc1(2 + si // 2)
        pend = (ph_ // 2, slice((ph_ % 2) * E, (ph_ % 2 + 1) * E), ph_, nrms)
        flush_pend()
        for ci in range(6, len(chunks)):
            emit_fc1(ci)
            emit_fc2(ci - 6)
        for ci in range(len(chunks) - 6, len(chunks)):
            emit_fc2(ci)
    if legalize:
        _legalize_matmul_waits(nc)
    return nc


_NC_CACHE = {}


def _get_nc():
    if "nc" not in _NC_CACHE:
        _NC_CACHE["nc"] = build()
    return _NC_CACHE["nc"]


def kernel(query, key, value, Wq, Wk, Wv, Wo, bo, **run_kwargs):
    query = np.asarray(query, dtype=np.float32)
    key = np.asarray(key, dtype=np.float32)
    value = np.asarray(value, dtype=np.float32)
    Wq = np.ascontiguousarray(np.asarray(Wq, dtype=np.float32))
    Wk = np.ascontiguousarray(np.asarray(Wk, dtype=np.float32))
    Wv = np.ascontiguousarray(np.asarray(Wv, dtype=np.float32))
    Wo = np.ascontiguousarray(np.asarray(Wo, dtype=np.float32))
    bo = np.ascontiguousarray(np.asarray(bo, dtype=np.float32))
    B = query.shape[0]
    assert B == 8, f"expected batch 8, got {B}"

    nc = _get_nc()
    in_maps = []
    for b in range(B):
        in_maps.append({
            "q": np.ascontiguousarray(query[b]),
            "k": np.ascontiguousarray(key[b]),
            "v": np.ascontiguousarray(value[b]),
            "wq": Wq, "wk": Wk, "wv": Wv, "wo": Wo, "bo": bo,
        })
    res = run_bass_kernel_spmd(nc, in_maps, core_ids=list(range(B)),
                               **run_kwargs)
    out = np.stack([r["out"] for r in res.results], axis=0)
    if run_kwargs.get("trace"):
        _NC_CACHE["last_result"] = res
    return out


# revision 7
# speedup vs baseline: 1.0775x; 1.0105x over previous
"""Multi-head attention Trainium2 kernel (nn_MultiHeadAttention_86423331930281).

Self-contained: data-parallel over batch (B=8 -> one batch element per
NeuronCore), runs on cores 0-7 via run_bass_kernel_spmd, returns the full
[8, 1024, 1024] output.

Per-core algorithm (S=1024, D=1024, H=16, E=64), all-bf16 matmul operands:
  - v/q/k: gpsimd cast-load fp32->bf16 two row-blocks per DMA, PE-transpose
    (bf16 identity, 1 cycle/row) into single [128, 8, S] tiles; one-bank
    row-block psum tiles give one evac per row (DVE/ACT alternating)
  - wo: gpsimd cast-load, store to DRAM bf16 scratch, xbar DMA-transpose
    back -> woT [he, out] (entirely off the critical path, SP-issued)
  - wv: gpsimd strided cast-load [ki, ko, h, e]; wq/wk: per-head-pair
    just-in-time gpsimd cast-loads, prefetched 3 pairs ahead
  - V1[t, h, e|1] = vT.T @ Wv with a trailing ones column per head
  - per head-pair: QT/KT [128=2*64, s] = Wq_pair-chunks.T @ qT (8-chunk
    accum in a dedicated psum pool, decoupled from the exp drain)
  - per head: scoresT [t, s] = KT_h-slices.T @ QT_h (K=64), exp on ACT
    (scale=1/32 folded) -> P [t, s] bf16
  - attended in [s, e] orientation with a ONE-HEAD SOFTWARE LAG: the
    previous head's chains att[s, 65] += P[t-chunk, s-chunk].T @ V1 are
    interleaved into the current head's score emission so they never wait
    on the serial exp stream; the 65th column accumulates the softmax
    denominator for free
  - normalize with per-partition reciprocal+multiply (denominator is a
    per-partition scalar in this orientation -- no broadcast round-trip),
    PE re-transpose [s,64]->[64,s] into attT [he, s]
  - FC split: FC1 = attT[m<7].T @ WoT runs during the last head's exp
    drain; partials (+bias, f32 DVE add) park in SBUF as bf16 and FC2's
    DVE evac adds them to the m=7 contribution -- no extra PE matmuls
"""

import numpy as np
from contextlib import ExitStack

import concourse.bass as bass
import concourse.mybir as mybir
import concourse.tile as tile
from concourse.bass_utils import run_bass_kernel_spmd
from concourse.masks import make_identity

P = 128
S = 1024          # sequence length
DK = 1024         # qkv input dim
H = 16            # heads
E = 64            # per-head dim
HE = H * E        # 1024
OUT = 1024        # output dim
NT = S // P       # 8 s/t tiles
NK = DK // P      # 8 contraction tiles
NM = H // 2       # 8 head pairs
F32 = mybir.dt.float32
BF16 = mybir.dt.bfloat16
AF = mybir.ActivationFunctionType
ALU = mybir.AluOpType
SCALE = 1.0 / 32.0  # 1/sqrt(DK)


def _legalize_matmul_waits(nc):
    """This walrus build allows only ONE sync-wait command per Matmult.
    Move all but the last wait of any multi-wait matmul onto freshly
    inserted PE nops immediately before it — same engine queue, so the
    blocking semantics are identical."""
    SKIP = ("NoOp", "Br", "Halt", "Sem", "Event")
    k = 0
    for f in nc.m.functions:
        for b in f.blocks:
            out = []
            for inst in b.instructions:
                si = getattr(inst, "sync_info", None)
                tname = type(inst).__name__
                if (not any(s in tname for s in SKIP) and si is not None
                        and si.on_wait and len(si.on_wait) > 1):
                    waits = list(si.on_wait)
                    for w in waits[:-1]:
                        nop = mybir.InstNoOp(
                            name=f"legalize-nop-{k}", ins=[], outs=[])
                        k += 1
                        nop.engine = inst.engine
                        nop.sync_info = mybir.SyncInfo(
                            on_wait=[w], on_update=[])
                        out.append(nop)
                    inst.sync_info = mybir.SyncInfo(
                        on_wait=[waits[-1]], on_update=list(si.on_update))
                out.append(inst)
            b.instructions[:] = out
    return k


def build(legalize=True):
    nc = bass.Bass()
    q_d = nc.dram_tensor("q", (S, DK), F32, kind="ExternalInput")
    k_d = nc.dram_tensor("k", (S, DK), F32, kind="ExternalInput")
    v_d = nc.dram_tensor("v", (S, DK), F32, kind="ExternalInput")
    wq_d = nc.dram_tensor("wq", (H, DK, E), F32, kind="ExternalInput")
    wk_d = nc.dram_tensor("wk", (H, DK, E), F32, kind="ExternalInput")
    wv_d = nc.dram_tensor("wv", (H, DK, E), F32, kind="ExternalInput")
    wo_d = nc.dram_tensor("wo", (OUT, HE), F32, kind="ExternalInput")
    bo_d = nc.dram_tensor("bo", (OUT,), F32, kind="ExternalInput")
    out_d = nc.dram_tensor("out", (S, OUT), F32, kind="ExternalOutput")
    wob_d = nc.dram_tensor("wob_scratch", (OUT, HE), BF16, kind="Internal")

    # [h, d, e] viewed as [di, ko, h, e] so partition = inner contraction dim
    wq_v = wq_d.rearrange("h (ko ki) e -> ki ko h e", ki=P)
    wk_v = wk_d.rearrange("h (ko ki) e -> ki ko h e", ki=P)
    wv_v = wv_d.rearrange("h (ko ki) e -> ki ko h e", ki=P)

    with tile.TileContext(nc) as tc, ExitStack() as ctx:
        const = ctx.enter_context(tc.tile_pool(name="const", bufs=1))
        src = ctx.enter_context(tc.tile_pool(name="src", bufs=4))
        xTq = ctx.enter_context(tc.tile_pool(name="xTq", bufs=1))
        woTp = ctx.enter_context(tc.tile_pool(name="woTp", bufs=NK))
        wqkp = ctx.enter_context(tc.tile_pool(name="wqkp", bufs=6))
        v1p = ctx.enter_context(tc.tile_pool(name="v1p", bufs=NT))
        # scores psum: 2 x 2 banks; proj/fc psum: 2 x 1 bank (decoupled so
        # projections never wait on the exp drain tail); the re-transpose
        # collect tiles share the proj slots (same tag, same bank size)
        scp = ctx.enter_context(tc.tile_pool(name="scp", bufs=2, space="PSUM"))
        pjp = ctx.enter_context(tc.tile_pool(name="pjp", bufs=2, space="PSUM"))
        ph1 = ExitStack()
        vTp = ph1.enter_context(tc.tile_pool(name="vTp", bufs=1))
        wvp = ph1.enter_context(tc.tile_pool(name="wvp", bufs=2))


        # ---- load + transpose phase --------------------------------------
        # one [128, NK, S] tile per transposed matrix; chunk j = [:, j, :]
        vTq = vTp.tile([P, NK, S], BF16, name="vT", tag="vT")
        qTq = xTq.tile([P, NK, S], BF16, name="qT", tag="qT")
        kTq = xTq.tile([P, NK, S], BF16, name="kT", tag="kT")
        woT = [woTp.tile([P, S], BF16, name=f"woT{j}", tag="woT")
               for j in range(NK)]

        tpp = ph1.enter_context(tc.tile_pool(name="tpp", bufs=2,
                                             space="PSUM"))

        NB = 2  # row-blocks per cast DMA (NB=4 halves the Pool dge count
        #         but crashes NRT execution; NB=2 is hardware-verified)

        def warmup(n):
            """dummy transposes keep the PE p-state ramp alive until the
            first real data lands (cost model runs full-rate only after
            3 us of continuous busy)"""
            dmy = tpp.tile([2, P], BF16, tag="tp", name="warm")
            for i in range(n):
                nc.tensor.transpose(dmy[0:2, :], ident_bf[:, 0:2],
                                    ident_bf[:])

        def emit_casts(mat_d, nm, n=None):
            """cast-load NB row-blocks per DMA"""
            stbs = []
            for rr in range(n if n is not None else NT // NB):
                stb = src.tile([P, NB, DK], BF16, tag="srcb",
                               name=f"{nm}cast{rr}")
                nc.gpsimd.dma_start(
                    stb[:],
                    mat_d[rr * NB * P:(rr + 1) * NB * P, :].rearrange(
                        "(c p) d -> p c d", c=NB))
                stbs.append(stb)
            return stbs

        def emit_transposes(stbs, tile, nm):
            """PE-transpose a full row-block into one 1-bank psum tile;
            single evac per row-block (DVE/ACT alternating — GPSIMD cannot
            read PSUM)"""
            for r in range(NT):
                stb = stbs[r // NB]
                c = r % NB
                # alternate with the (still idle) scores pool slots so the
                # evac WAR never paces the transposes
                pool, tg = (tpp, "tp") if r % 2 == 0 else (scp, "sc")
                pt_ = pool.tile([P, NK, P], BF16, tag=tg, name=f"{nm}ps{r}")
                for j in range(NK):
                    nc.tensor.transpose(
                        pt_[:, j, :], stb[:, c, j * P:(j + 1) * P],
                        ident_bf[:])
                if r % 2 == 0:
                    nc.vector.tensor_copy(
                        tile[:, :, r * P:(r + 1) * P], pt_[:])
                else:
                    nc.scalar.copy(
                        tile[:, :, r * P:(r + 1) * P], pt_[:])

        def pe_transpose(mat_d, tile, nm):
            emit_transposes(emit_casts(mat_d, nm), tile, nm)

        def xs(tile, j):
            """[128, S] view of transposed chunk j"""
            return tile[:, j, :]

        # first v cast goes out before the identity init so data and
        # identity land together for the first transpose
        v_stbs = emit_casts(v_d[0:NB * P, :], "v0", n=1)
        ident = const.tile([P, P], F32, name="ident")
        make_identity(nc, ident)
        ident_bf = const.tile([P, P], BF16, name="ident_bf")
        nc.vector.tensor_copy(ident_bf[:], ident[:])
        v_stbs += emit_casts(v_d[NB * P:, :], "v1", n=3)
        warmup(56)
        emit_transposes(v_stbs, vTq, "v")

        # wv: strided cast-load [ki, ko, h, e] per contraction chunk
        wv_sb = []
        for half in range(2):
            t = wvp.tile([P, NK // 2, H, E], BF16, tag="wwv",
                         name=f"wvsb{half}")
            for jj in range(NK // 2):
                nc.gpsimd.dma_start(
                    t[:, jj], wv_v[:, half * (NK // 2) + jj])
            wv_sb.append(t)

        def prefetch_w(m):
            """per-pair just-in-time Wq/Wk chunk loads [ki, ko, 2, e]"""
            wqm = wqkp.tile([P, NK, 2, E], BF16, tag="wqk", name=f"wqm{m}")
            wkm = wqkp.tile([P, NK, 2, E], BF16, tag="wqk", name=f"wkm{m}")
            for hh in range(2):
                nc.gpsimd.dma_start(wqm[:, :, hh, :], wq_v[:, :, 2 * m + hh, :])
                nc.gpsimd.dma_start(wkm[:, :, hh, :], wk_v[:, :, 2 * m + hh, :])
            return wqm, wkm

        w_pref = {0: prefetch_w(0)}

        # ---- V projection: V1 [t, h, e|ones] ------------------------------
        v1_tiles = []
        for i in range(NT):
            v1 = v1p.tile([P, H, E + 1], BF16, tag="v1", name=f"v1_{i}")
            nc.gpsimd.memset(v1[:, :, E], 1.0)
            for nh in range(2):
                pst = pjp.tile([P, 512], F32, tag="pj", name=f"vproj{i}_{nh}")
                for j in range(NK):
                    wvf = wv_sb[j // (NK // 2)][:, j % (NK // 2)].rearrange(
                        "p h e -> p (h e)")
                    nc.tensor.matmul(
                        pst[:],
                        xs(vTq, j)[:, i * P:(i + 1) * P],
                        wvf[:, nh * 512:(nh + 1) * 512],
                        start=(j == 0), stop=(j == NK - 1))
                nc.vector.tensor_copy(
                    v1[:, nh * (H // 2):(nh + 1) * (H // 2), 0:E],
                    pst[:].rearrange("p (h e) -> p h e", e=E))
            v1_tiles.append(v1)

        w_pref[1] = prefetch_w(1)
        pe_transpose(q_d, qTq, "q")
        w_pref[2] = prefetch_w(2)
        pe_transpose(k_d, kTq, "k")

        # FC-only constant, emitted after the critical-path loads (SP HWDGE)
        bo_bc = const.tile([P, OUT], F32, name="bo_bc")
        nc.sync.dma_start(bo_bc[:], bo_d[None, :].to_broadcast((P, OUT)))

        ph1.close()

        # ---- attention (one-head software pipeline lag) -------------------
        qtp = ctx.enter_context(tc.tile_pool(name="qtp", bufs=4))
        ptp = ctx.enter_context(tc.tile_pool(name="ptp", bufs=2 * NT))
        normp = ctx.enter_context(tc.tile_pool(name="normp", bufs=16))
        denp = ctx.enter_context(tc.tile_pool(name="denp", bufs=8))
        attp = ctx.enter_context(tc.tile_pool(name="attp", bufs=NM))
        att_ps = ctx.enter_context(
            tc.tile_pool(name="att_ps", bufs=2, space="PSUM"))

        attT_tiles = [attp.tile([P, S], BF16, tag="attT", name=f"attT{m}")
                      for m in range(NM)]

        # wo: cast-load bf16 (gpsimd, queued after the critical-path loads),
        # store to scratch (SP), xbar DMA-transpose back (SP) — SP is
        # otherwise idle until the out writes; needed only by the FC
        for rr in range(NT // 2):
            stb = src.tile([P, 2, DK], BF16, tag="srcb", name=f"wocast{rr}")
            nc.gpsimd.dma_start(
                stb[:],
                wo_d[rr * 2 * P:(rr + 1) * 2 * P, :].rearrange(
                    "(c p) d -> p c d", c=2))
            for c in range(2):
                r = rr * 2 + c
                nc.sync.dma_start(wob_d[r * P:(r + 1) * P, :], stb[:, c, :])
        for j in range(NK):
            nc.sync.dma_start_transpose(
                woT[j][:], wob_d[:, j * P:(j + 1) * P])

        def emit_att(h, ptiles, si):
            """attended [s-chunk si, e|denom] for head h + normalize"""
            if si % 2 == 0:
                _att_slot[0] = att_ps.tile([P, 2, E + 1], F32, tag="attps",
                                           name=f"att{h}_{si}")
            aps = _att_slot[0][:, si % 2, :]
            for j in range(NT):
                nc.tensor.matmul(
                    aps[0:P, 0:E + 1],
                    ptiles[j][:, si * P:(si + 1) * P],
                    v1_tiles[j][:, h, :],
                    start=(j == 0), stop=(j == NT - 1))
            den = denp.tile([P, 1], F32, tag="den", name=f"den{h}_{si}")
            nc.vector.reciprocal(den[:], aps[0:P, E:E + 1])
            nrm = normp.tile([P, E], BF16, tag="nrm", name=f"nrm{h}_{si}")
            nc.vector.tensor_scalar(nrm[:], aps[0:P, 0:E], den[:], None,
                                    ALU.mult)
            return nrm

        _att_slot = [None]
        pend = None  # (m, hs, nrm list) awaiting re-transpose + evac

        def flush_pend():
            nonlocal pend
            if pend is None:
                return
            pm, phs, ph_, nrms = pend
            tph = pjp.tile([E, S], BF16, tag="pj", name=f"tph{ph_}")
            for si in range(NT):
                nc.tensor.transpose(tph[:, si * P:(si + 1) * P], nrms[si][:],
                                    ident_bf[:])
            nc.vector.tensor_copy(attT_tiles[pm][phs, :], tph[:])
            pend = None

        prev_att = None  # (h, ptiles) whose attended chains interleave next

        for m in range(NM):
            if m + 3 < NM:
                w_pref[m + 3] = prefetch_w(m + 3)
            wqm, wkm = w_pref.pop(m)

            # QT_m / KT_m: [he_pair=128, s=1024], evacuated as bf16
            qkm = []
            for wm, xtiles, lbl in ((wqm, qTq, "qtm"), (wkm, kTq, "ktm")):
                t = qtp.tile([P, S], BF16, tag="qt", name=f"{lbl}{m}")
                for sh in range(2):
                    pst = pjp.tile([P, 512], F32, tag="pj",
                                   name=f"{lbl}ps{m}_{sh}")
                    for j in range(NK):
                        nc.tensor.matmul(
                            pst[:],
                            wm[:, j],
                            xs(xtiles, j)[:, sh * 512:(sh + 1) * 512],
                            start=(j == 0), stop=(j == NK - 1))
                    nc.vector.tensor_copy(t[:, sh * 512:(sh + 1) * 512],
                                          pst[:])
                qkm.append(t)
            qtm, ktm = qkm

            for hh in range(2):
                h = 2 * m + hh
                hs = slice(hh * E, (hh + 1) * E)
                # scoresT + exp -> P_j [t, s] bf16, with the previous head's
                # attended chains interleaved (their exps are already done)
                ptiles = []
                for j in range(NT):
                    pt = ptp.tile([P, S], BF16, tag="pt", name=f"p{h}_{j}")
                    sc = scp.tile([P, S], F32, tag="sc", name=f"sc{h}_{j}")
                    for sh in range(2):
                        nc.tensor.matmul(
                            sc[:, sh * 512:(sh + 1) * 512],
                            ktm[hs, j * P:(j + 1) * P],
                            qtm[hs, sh * 512:(sh + 1) * 512],
                            start=True, stop=True)
                    nc.scalar.activation(pt[:], sc[:], AF.Exp, scale=SCALE)
                    ptiles.append(pt)
                    if prev_att is not None:
                        nrm = emit_att(prev_att[0], prev_att[1], j)
                        prev_att[2].append(nrm)
                if prev_att is not None:
                    ph_, ppt, nrms = prev_att
                    flush_pend()
                    pend = (ph_ // 2, slice((ph_ % 2) * E, (ph_ % 2 + 1) * E),
                            ph_, nrms)
                prev_att = (h, ptiles, [])

        # ---- drain + FC, software-pipelined ------------------------------
        # FC1 (heads of pairs 0..6) runs while the last head's exps drain;
        # its partials (+bias) park in SBUF. The tail is then only the last
        # head's attended, its transposes, and a single-matmul FC2 pass.
        outp = ctx.enter_context(tc.tile_pool(name="outp", bufs=8))
        fc1p = ctx.enter_context(tc.tile_pool(name="fc1p", bufs=2 * NT))
        ph_, ppt, nrms = prev_att
        flush_pend()
        chunks = [(st, oh) for st in range(NT) for oh in range(2)]
        fc1_sb = [None] * len(chunks)

        def emit_fc1(ci):
            st, oh = chunks[ci]
            # 4-slot rotation with FC2 across both psum pools (scores pool
            # is idle during the drain) so evac latency never gates PE
            pool, tag = (pjp, "pj") if ci % 2 == 0 else (scp, "sc")
            pso = pool.tile([P, 512], F32, tag=tag, name=f"fc1_{st}_{oh}")
            for m in range(NM - 1):
                nc.tensor.matmul(
                    pso[:],
                    attT_tiles[m][:, st * P:(st + 1) * P],
                    woT[m][:, oh * 512:(oh + 1) * 512],
                    start=(m == 0), stop=(m == NM - 2))
            t = fc1p.tile([P, 512], BF16, tag="fc1", name=f"fc1sb{st}_{oh}")
            # DVE evac folds the bias in; the last head's normalize chain is
            # already drained by the early att emission, so no hostage stall
            nc.vector.tensor_tensor(
                t[:], pso[:], bo_bc[:, oh * 512:(oh + 1) * 512], ALU.add)
            fc1_sb[ci] = t

        def emit_fc2(ci):
            st, oh = chunks[ci]
            pool, tag = (scp, "sc") if ci % 2 == 0 else (pjp, "pj")
            pso = pool.tile([P, 512], F32, tag=tag, name=f"fc2_{st}_{oh}")
            nc.tensor.matmul(
                pso[:],
                attT_tiles[NM - 1][:, st * P:(st + 1) * P],
                woT[NM - 1][:, oh * 512:(oh + 1) * 512],
                start=True, stop=True)
            ot = outp.tile([P, 512], F32, tag="out", name=f"out{st}_{oh}")
            nc.vector.tensor_tensor(ot[:], pso[:], fc1_sb[ci][:], ALU.add)
            nc.sync.dma_start(
                out_d[st * P:(st + 1) * P, oh * 512:(oh + 1) * 512], ot[:])

        # the last head's attended chains go out immediately (their exps
        # finish during the first FC1 chunk), so attT completes early and
        # the serialized out-DMA stream can start near the drain's front
        emit_fc1(0)
        emit_fc1(1)
        for si in range(NT):
            nrms.append(emit_att(ph_, ppt, si))
            if si % 2 == 1 and 2 + si // 2 < len(chunks):
                emit_fc1(2 + si // 2)
        pend = (ph_ // 2, slice((ph_ % 2) * E, (ph_ % 2 + 1) * E), ph_, nrms)
        flush_pend()
        for ci in range(6, len(chunks)):
            emit_fc1(ci)
            emit_fc2(ci - 6)
        for ci in range(len(chunks) - 6, len(chunks)):
            emit_fc2(ci)
    if legalize:
        _legalize_matmul_waits(nc)
    return nc


_NC_CACHE = {}


def _get_nc():
    if "nc" not in _NC_CACHE:
        _NC_CACHE["nc"] = build()
    return _NC_CACHE["nc"]


def kernel(query, key, value, Wq, Wk, Wv, Wo, bo, **run_kwargs):
    query = np.asarray(query, dtype=np.float32)
    key = np.asarray(key, dtype=np.float32)
    value = np.asarray(value, dtype=np.float32)
    Wq = np.ascontiguousarray(np.asarray(Wq, dtype=np.float32))
    Wk = np.ascontiguousarray(np.asarray(Wk, dtype=np.float32))
    Wv = np.ascontiguousarray(np.asarray(Wv, dtype=np.float32))
    Wo = np.ascontiguousarray(np.asarray(Wo, dtype=np.float32))
    bo = np.ascontiguousarray(np.asarray(bo, dtype=np.float32))
    B = query.shape[0]
    assert B == 8, f"expected batch 8, got {B}"

    nc = _get_nc()
    in_maps = []
    for b in range(B):
        in_maps.append({
            "q": np.ascontiguousarray(query[b]),
            "k": np.ascontiguousarray(key[b]),
            "v": np.ascontiguousarray(value[b]),
            "wq": Wq, "wk": Wk, "wv": Wv, "wo": Wo, "bo": bo,
        })
    res = run_bass_kernel_spmd(nc, in_maps, core_ids=list(range(B)),
                               **run_kwargs)
    out = np.stack([r["out"] for r in res.results], axis=0)
    if run_kwargs.get("trace"):
        _NC_CACHE["last_result"] = res
    return out
